# revision 1
# baseline (speedup 1.0000x reference)
"""Multi-head attention (B=2, N=2048, C=768, H=12, DH=64) on 8 Trainium2 cores.

Sharding: data-parallel on batch (cores 0-3 -> b=0, cores 4-7 -> b=1),
tensor-parallel on heads within each group (3 heads/core: Wq/Wk/Wv column
slices, Wp row slices).  Each core emits its partial projection output
[N, C]; the host sums the 4 partials per batch and adds bp (cheaper than a
device collective at this size).

Per-core dataflow (feature-major, transpose-free, fp16 operands / fp32 psum):
  - host supplies xT = x[b].T  [C, N] in fp16
  - qT,kT [64, N] per head = W.T @ xT       (W natural layout as lhsT)
  - v     [N, 192] token-major from xT as lhsT, with a ones column per head
  - ST    [kj, qi] = kT.T-slice @ qT        (scores, transposed); two K=64
    matmuls packed on disjoint PE row halves per [128,1024] psum tile
    (heads 0+1 paired; head 2 pairs even/odd kj via partition-duplicated k/q)
  - ET    = exp(ST - 4) one ACT op per [128,1024]  (shift cancels in softmax)
  - yT_aug[65, qi] = [v_h | 1].T @ ET accumulated over kj; row 64 = denominator
  - software pipeline: ST(kj+1) issues before yT(kj) so PE never waits on ACT
  - normalize: reciprocal of denom row, stride-0 DMA broadcast, fused mul-copy
  - out[qi, C] partial = yT (stationary) @ Wp rows: K=128 (heads 0+1) + K=64
"""

import math
import os

import ml_dtypes
import numpy as np

import concourse.bacc as bacc
import concourse.bass as bass
import concourse.mybir as mybir
import concourse.tile as tile
from concourse import bass_utils

B, N, C, H, DH = 2, 2048, 768, 12, 64
NCORES = 8
CPG = 4                  # cores per batch group
HPC = H // CPG           # heads per core = 3
MYC = HPC * DH           # per-core feature width = 192
KC = C // 128            # contraction chunks = 6
NTT = N // 128           # token tiles = 16
QB = 512                 # qi block (psum bank width, fp32)
F32 = mybir.dt.float32
MMDT = mybir.dt.float16  # matmul operand dtype: 1cyc/row, 10-bit mantissa
AF = mybir.ActivationFunctionType
OP = mybir.AluOpType

EXP_SHIFT = -4.0         # exp(s + EXP_SHIFT); cancels between num and denom


def _bcast_parts(ap, nparts):
    """Partition-stride-0 broadcast view of a [1, F] AP (DMA source only)."""
    return bass.AP(tensor=ap.tensor, offset=ap.offset,
                   ap=[[0, nparts]] + [list(d) for d in ap.ap[1:]])


def _emit(nc, tc, pools, aps):
    xT, wq, wk, wv, wp, bq, bk, bv, out = (
        aps["xT"], aps["wq"], aps["wk"], aps["wv"], aps["wp"],
        aps["bq"], aps["bk"], aps["bv"], aps["out"],
    )
    persist = pools["persist"]
    et_pool = pools["et"]
    small = pools["small"]
    ostage = pools["ostage"]

    # ---- persistent SBUF tensors ----
    xT_sb = persist.tile([128, KC * N], MMDT, tag="xT_sb")
    wq_sb = persist.tile([128, KC * MYC], MMDT, tag="wq_sb")
    wk_sb = persist.tile([128, KC * MYC], MMDT, tag="wk_sb")
    wv_sb = persist.tile([128, KC * MYC], MMDT, tag="wv_sb")
    wpA = persist.tile([128, C], MMDT, tag="wpA")
    wpB = persist.tile([64, C], MMDT, tag="wpB")
    bqA = persist.tile([128, 1], F32, tag="bqA")
    bqB = persist.tile([64, 1], F32, tag="bqB")
    bkA = persist.tile([128, 1], F32, tag="bkA")
    bkB = persist.tile([64, 1], F32, tag="bkB")
    bv_row = persist.tile([1, MYC], MMDT, tag="bv_row")
    ones = persist.tile([1, 128], MMDT, tag="ones")
    shift_col = persist.tile([128, 1], F32, tag="shift_col")
    qTA = persist.tile([128, N], MMDT, tag="qTA")
    kTA = persist.tile([128, N], MMDT, tag="kTA")
    # head 2 k/q live duplicated on both partition halves (kj even/odd packing)
    qTB = persist.tile([128, N], MMDT, tag="qTB")
    kTB = persist.tile([128, N], MMDT, tag="kTB")
    v_sb = persist.tile([128, NTT * HPC * 65], MMDT, tag="v_sb")
    yTA = persist.tile([128, N], MMDT, tag="yTA")
    yTB = persist.tile([64, N], MMDT, tag="yTB")

    # ---- input DMAs ----
    for kc in range(KC):
        nc.sync.dma_start(out=xT_sb[:, kc * N:(kc + 1) * N],
                          in_=xT[kc * 128:(kc + 1) * 128, :])
        nc.sync.dma_start(out=wq_sb[:, kc * MYC:(kc + 1) * MYC],
                          in_=wq[kc * 128:(kc + 1) * 128, :])
        nc.sync.dma_start(out=wk_sb[:, kc * MYC:(kc + 1) * MYC],
                          in_=wk[kc * 128:(kc + 1) * 128, :])
        nc.sync.dma_start(out=wv_sb[:, kc * MYC:(kc + 1) * MYC],
                          in_=wv[kc * 128:(kc + 1) * 128, :])
    nc.sync.dma_start(out=wpA, in_=wp[0:128, :])
    nc.sync.dma_start(out=wpB, in_=wp[128:MYC, :])
    nc.sync.dma_start(out=bqA, in_=bq[0:128, :])
    nc.sync.dma_start(out=bqB, in_=bq[128:MYC, :])
    nc.sync.dma_start(out=bkA, in_=bk[0:128, :])
    nc.sync.dma_start(out=bkB, in_=bk[128:MYC, :])
    nc.sync.dma_start(out=bv_row, in_=bv)
    ones_f32 = persist.tile([128, 1], F32, tag="ones_f32")
    ones_row_f32 = persist.tile([1, 128], F32, tag="ones_row_f32")
    nc.vector.memset(ones_f32, 1.0)
    nc.vector.memset(ones_row_f32, 1.0)
    nc.vector.tensor_copy(out=ones, in_=ones_row_f32)
    nc.vector.memset(shift_col, EXP_SHIFT)

    # ---- phases 1+2: q/k/v projections (own PSUM pool, released after) ----
    with tc.tile_pool(name="ps_proj", bufs=2, space="PSUM") as ps_proj:
        for wsb, dstA, dstB, biasA, biasB in (
            (wq_sb, qTA, qTB, bqA, bqB),
            (wk_sb, kTA, kTB, bkA, bkB),
        ):
            for mg in range(2):
                M = 128 if mg == 0 else 64
                dst = dstA if mg == 0 else dstB
                bias = biasA if mg == 0 else biasB
                pss = [ps_proj.tile([M, QB], F32, tag="ps_qk", bufs=5,
                                    name=f"ps_qk{_i}")
                       for _i in range(N // QB)]
                for kc in range(KC):  # kc outer: overlap the xT load
                    for nt in range(N // QB):
                        nc.tensor.matmul(
                            pss[nt],
                            wsb[:, kc * MYC + mg * 128: kc * MYC + mg * 128 + M],
                            xT_sb[:, kc * N + nt * QB: kc * N + nt * QB + QB],
                            start=(kc == 0), stop=(kc == KC - 1),
                        )
                for nt in range(N // QB):
                    nc.vector.tensor_scalar(
                        out=dst[0:M, nt * QB:(nt + 1) * QB], in0=pss[nt],
                        scalar1=bias[0:M, :], scalar2=None, op0=OP.add,
                    )
        # duplicate head-2 k/q onto partitions 64..127 (cross-partition: DMA)
        nc.sync.dma_start(out=qTB[64:128, :], in_=qTB[0:64, :])
        nc.sync.dma_start(out=kTB[64:128, :], in_=kTB[0:64, :])

        for nt in range(NTT):
            ps = ps_proj.tile([128, MYC], F32, tag="ps_v")
            for kc in range(KC):
                nc.tensor.matmul(
                    ps,
                    xT_sb[:, kc * N + nt * 128: kc * N + nt * 128 + 128],
                    wv_sb[:, kc * MYC:(kc + 1) * MYC],
                    start=(kc == 0), stop=False,
                )
            nc.tensor.matmul(ps, ones[0:1, 0:128], bv_row,
                             start=False, stop=True)
            for h in range(HPC):
                base = (nt * HPC + h) * 65
                nc.vector.tensor_copy(out=v_sb[:, base:base + 64],
                                      in_=ps[:, h * 64:(h + 1) * 64])
                nc.vector.tensor_copy(out=v_sb[:, base + 64:base + 65],
                                      in_=ones_f32)

    phases = os.environ.get("K_PHASES", "1234")
    if "3" not in phases:
        for i, src_t in enumerate((qTA, kTA, qTB, v_sb)):
            dump = ostage.tile([128, C], F32, name=f"dump{i}")
            nc.vector.tensor_copy(out=dump, in_=src_t[:, 0:C])
            nc.sync.dma_start(out=out[i * 128:(i + 1) * 128, :], in_=dump)
        return

    # ---- phase 3: attention; unit = (head-pair, qi block of 512) ----
    def vh_ap(kj, h):
        base = (kj * HPC + h) * 65
        return v_sb[:, base:base + 65]

    dram_bc = pools["dram_bc"]

    def normalize(yt, ydst, q0):
        rec = small.tile([1, QB], F32, tag="rec")
        nc.vector.reciprocal(rec, yt[64:65, :])
        dr = dram_bc.tile([1, QB], F32)
        nc.sync.dma_start(out=dr, in_=rec)
        bc = small.tile([64, QB], F32, tag="bc_sb")
        nc.sync.dma_start(out=bc, in_=_bcast_parts(dr, 64))
        nc.vector.scalar_tensor_tensor(
            out=ydst[:, q0:q0 + QB], in0=yt[0:64, :], scalar=1.0, in1=bc,
            op0=OP.mult, op1=OP.mult,
        )

    def proj_block(ps_st, qq):
        # projection for qi tiles of block qq; psum carved from st-pool slots
        # (two [128,384] outs in the two banks of one [128,1024] slot)
        for qt in range(qq * 4, qq * 4 + 4):
            stt = ps_st.tile([128, 1024], F32, tag="st", name=f"pj{qt}")
            ob = ostage.tile([128, C], F32, name=f"ob{qt}")
            for nb in range(2):
                po = stt[:, nb * QB: nb * QB + 384]
                nc.tensor.matmul(po, yTA[:, qt * 128:(qt + 1) * 128],
                                 wpA[:, nb * 384:(nb + 1) * 384],
                                 start=True, stop=False)
                nc.tensor.matmul(po, yTB[0:64, qt * 128:(qt + 1) * 128],
                                 wpB[0:64, nb * 384:(nb + 1) * 384],
                                 start=False, stop=True)
                nc.vector.tensor_copy(out=ob[:, nb * 384:(nb + 1) * 384],
                                      in_=po)
            nc.sync.dma_start(out=out[qt * 128:(qt + 1) * 128, :], in_=ob)

    with tc.tile_pool(name="ps_st", bufs=2, space="PSUM") as ps_st, \
         tc.tile_pool(name="ps_yt", bufs=4, space="PSUM") as ps_yt:
        for qq in range(4):
            q0 = qq * QB

            # --- head 2, even/odd kj pairs on the PE array halves ---
            yt2 = ps_yt.tile([65, QB], F32, tag="yt")
            prev = None
            for kp in range(NTT // 2):
                kj0, kj1 = 2 * kp, 2 * kp + 1
                st = ps_st.tile([128, 1024], F32, tag="st")
                nc.tensor.matmul(st[:, 0:QB],
                                 kTB[0:64, kj0 * 128:(kj0 + 1) * 128],
                                 qTB[0:64, q0:q0 + QB], start=True, stop=True)
                nc.tensor.matmul(st[:, QB:1024],
                                 kTB[64:128, kj1 * 128:(kj1 + 1) * 128],
                                 qTB[64:128, q0:q0 + QB], start=True, stop=True)
                et = et_pool.tile([128, 1024], MMDT)
                nc.scalar.activation(et, st, AF.Exp, bias=shift_col[:, :])
                if prev is not None:
                    pet, pkp = prev
                    nc.tensor.matmul(yt2, vh_ap(2 * pkp, 2), pet[:, 0:QB],
                                     start=(pkp == 0), stop=False)
                    nc.tensor.matmul(yt2, vh_ap(2 * pkp + 1, 2),
                                     pet[:, QB:1024], start=False, stop=False)
                prev = (et, kp)
            pet, pkp = prev
            nc.tensor.matmul(yt2, vh_ap(2 * pkp, 2), pet[:, 0:QB],
                             start=(pkp == 0), stop=False)
            nc.tensor.matmul(yt2, vh_ap(2 * pkp + 1, 2), pet[:, QB:1024],
                             start=False, stop=True)
            normalize(yt2, yTB[0:64, :], q0)

            # --- heads 0+1, row-paired on the PE array ---
            yt0 = ps_yt.tile([65, QB], F32, tag="yt")
            yt1 = ps_yt.tile([65, QB], F32, tag="yt")
            prev = None
            for kj in range(NTT):
                st = ps_st.tile([128, 1024], F32, tag="st")
                nc.tensor.matmul(st[:, 0:QB],
                                 kTA[0:64, kj * 128:(kj + 1) * 128],
                                 qTA[0:64, q0:q0 + QB], start=True, stop=True)
                nc.tensor.matmul(st[:, QB:1024],
                                 kTA[64:128, kj * 128:(kj + 1) * 128],
                                 qTA[64:128, q0:q0 + QB], start=True, stop=True)
                et = et_pool.tile([128, 1024], MMDT)
                nc.scalar.activation(et, st, AF.Exp, bias=shift_col[:, :])
                if prev is not None:
                    pet, pkj = prev
                    nc.tensor.matmul(yt0, vh_ap(pkj, 0), pet[:, 0:QB],
                                     start=(pkj == 0), stop=False)
                    nc.tensor.matmul(yt1, vh_ap(pkj, 1), pet[:, QB:1024],
                                     start=(pkj == 0), stop=False)
                prev = (et, kj)
            pet, pkj = prev
            nc.tensor.matmul(yt0, vh_ap(pkj, 0), pet[:, 0:QB],
                             start=False, stop=True)
            nc.tensor.matmul(yt1, vh_ap(pkj, 1), pet[:, QB:1024],
                             start=False, stop=True)
            normalize(yt0, yTA[0:64, :], q0)
            normalize(yt1, yTA[64:128, :], q0)

            # projection for the PREVIOUS block overlaps this block's drain
            if os.environ.get("K_PROJ", "fused") == "fused":
                if qq > 0:
                    proj_block(ps_st, qq - 1)
        if os.environ.get("K_PROJ", "fused") == "fused":
            proj_block(ps_st, 3)
        else:
            for qq in range(4):
                proj_block(ps_st, qq)




def _build_program():
    nc = bacc.Bacc("TRN2", target_bir_lowering=False, debug=False,
                   num_devices=NCORES)
    aps = {
        "xT": nc.dram_tensor("xT", [C, N], MMDT, kind="ExternalInput").ap(),
        "wq": nc.dram_tensor("wq", [C, MYC], MMDT, kind="ExternalInput").ap(),
        "wk": nc.dram_tensor("wk", [C, MYC], MMDT, kind="ExternalInput").ap(),
        "wv": nc.dram_tensor("wv", [C, MYC], MMDT, kind="ExternalInput").ap(),
        "wp": nc.dram_tensor("wp", [MYC, C], MMDT, kind="ExternalInput").ap(),
        "bq": nc.dram_tensor("bq", [MYC, 1], F32, kind="ExternalInput").ap(),
        "bk": nc.dram_tensor("bk", [MYC, 1], F32, kind="ExternalInput").ap(),
        "bv": nc.dram_tensor("bv", [1, MYC], MMDT, kind="ExternalInput").ap(),
        "out": nc.dram_tensor("out", [N, C], F32, kind="ExternalOutput").ap(),
    }
    with tile.TileContext(nc) as tc:
        import contextlib
        with contextlib.ExitStack() as ctx:
            pools = {
                "persist": ctx.enter_context(tc.tile_pool(name="persist", bufs=1)),
                "et": ctx.enter_context(tc.tile_pool(name="et", bufs=3)),
                "small": ctx.enter_context(tc.tile_pool(name="small", bufs=2)),
                "ostage": ctx.enter_context(tc.tile_pool(name="ostage", bufs=2)),
                "dram_bc": ctx.enter_context(
                    tc.tile_pool(name="dram_bc", bufs=2, space="DRAM")),
            }
            _emit(nc, tc, pools, aps)
    nc.compile()
    return nc


_PROGRAM_CACHE = {}


def _get_program():
    if "nc" not in _PROGRAM_CACHE:
        _PROGRAM_CACHE["nc"] = _build_program()
    return _PROGRAM_CACHE["nc"]


def make_in_maps(x, Wq, bq, Wk, bk, Wv, bv, Wp, bp):
    scale = 1.0 / math.sqrt(DH)
    xTb = [np.ascontiguousarray(x[b].T) for b in range(B)]
    wire = mybir.dt.np(MMDT)
    in_maps = []
    for c in range(NCORES):
        b, hg = c // CPG, c % CPG
        cols = slice(hg * MYC, (hg + 1) * MYC)
        in_maps.append({
            "xT": xTb[b].astype(wire),
            "wq": (np.ascontiguousarray(Wq[:, cols]) * np.float32(scale)).astype(wire),
            "wk": np.ascontiguousarray(Wk[:, cols]).astype(wire),
            "wv": np.ascontiguousarray(Wv[:, cols]).astype(wire),
            "wp": np.ascontiguousarray(Wp[cols, :]).astype(wire),
            "bq": (bq[cols] * np.float32(scale)).reshape(MYC, 1).copy(),
            "bk": bk[cols].reshape(MYC, 1).copy(),
            "bv": bv[cols].reshape(1, MYC).astype(wire),
        })
    return in_maps


def assemble(results, bp):
    out = np.empty((B, N, C), np.float32)
    for b in range(B):
        acc = results[b * CPG]["out"].astype(np.float64)
        for c in range(b * CPG + 1, (b + 1) * CPG):
            acc = acc + results[c]["out"]
        out[b] = (acc + bp.astype(np.float64)).astype(np.float32)
    return out


def kernel(x, Wq, bq, Wk, bk, Wv, bv, Wp, bp, **extra_kwargs):
    x = np.asarray(x, np.float32)
    Wq = np.asarray(Wq, np.float32)
    Wk = np.asarray(Wk, np.float32)
    Wv = np.asarray(Wv, np.float32)
    Wp = np.asarray(Wp, np.float32)
    bq = np.asarray(bq, np.float32)
    bk = np.asarray(bk, np.float32)
    bv = np.asarray(bv, np.float32)
    bp = np.asarray(bp, np.float32)

    nc = _get_program()
    in_maps = make_in_maps(x, Wq, bq, Wk, bk, Wv, bv, Wp, bp)
    res = bass_utils.run_bass_kernel_spmd(nc, in_maps,
                                          core_ids=list(range(NCORES)))
    return assemble(res.results, bp)



# revision 4
# speedup vs baseline: 1.1200x; 1.1200x over previous
"""Multi-head attention (B=2, N=2048, C=768, H=12, DH=64) on 8 Trainium2 cores.

Sharding: data-parallel on batch (cores 0-3 -> b=0, cores 4-7 -> b=1),
tensor-parallel on heads within each group (3 heads/core: Wq/Wk/Wv column
slices, Wp row slices).  Each core emits its partial projection output
[N, C]; the host sums the 4 partials per batch and adds bp (cheaper than a
device collective at this size).

Per-core dataflow (feature-major, transpose-free, fp16 operands / fp32 psum):
  - host supplies xT = x[b].T  [C, N] in fp16
  - qT,kT [64, N] per head = W.T @ xT       (W natural layout as lhsT)
  - v     [N, 192] token-major from xT as lhsT, with a ones column per head
  - ST    [kj, qi] = kT.T-slice @ qT        (scores, transposed); two K=64
    matmuls packed on disjoint PE row halves per [128,1024] psum tile
    (heads 0+1 paired; head 2 pairs even/odd kj via partition-duplicated k/q)
  - ET    = exp(ST - 4) one ACT op per [128,1024]  (shift cancels in softmax)
  - yT_aug[65, qi] = [v_h | 1].T @ ET accumulated over kj; row 64 = denominator
  - software pipeline: ST(kj+1) issues before yT(kj) so PE never waits on ACT
  - normalize: reciprocal of denom row, stride-0 DMA broadcast, fused mul-copy
  - out[qi, C] partial = yT (stationary) @ Wp rows: K=128 (heads 0+1) + K=64
"""

import math
import os

import ml_dtypes
import numpy as np

import concourse.bacc as bacc
import concourse.bass as bass
import concourse.mybir as mybir
import concourse.tile as tile
from concourse import bass_utils

B, N, C, H, DH = 2, 2048, 768, 12, 64
NCORES = 8
CPG = 4                  # cores per batch group
HPC = H // CPG           # heads per core = 3
MYC = HPC * DH           # per-core feature width = 192
KC = C // 128            # contraction chunks = 6
NTT = N // 128           # token tiles = 16
QB = 512                 # qi block (psum bank width, fp32)
F32 = mybir.dt.float32
MMDT = mybir.dt.float16  # matmul operand dtype: 1cyc/row, 10-bit mantissa
AF = mybir.ActivationFunctionType
OP = mybir.AluOpType

EXP_SHIFT = -4.0         # exp(s + EXP_SHIFT); cancels between num and denom


def _bcast_parts(ap, nparts):
    """Partition-stride-0 broadcast view of a [1, F] AP (DMA source only)."""
    return bass.AP(tensor=ap.tensor, offset=ap.offset,
                   ap=[[0, nparts]] + [list(d) for d in ap.ap[1:]])


def _emit(nc, tc, pools, aps):
    xT, wq, wk, wv, wp, bq, bk, bv, out = (
        aps["xT"], aps["wq"], aps["wk"], aps["wv"], aps["wp"],
        aps["bq"], aps["bk"], aps["bv"], aps["out"],
    )
    persist = pools["persist"]
    et_pool = pools["et"]
    small = pools["small"]
    ostage = pools["ostage"]

    # ---- persistent SBUF tensors ----
    xT_sb = persist.tile([128, KC * N], MMDT, tag="xT_sb")
    wq_sb = persist.tile([128, KC * MYC], MMDT, tag="wq_sb")
    wk_sb = persist.tile([128, KC * MYC], MMDT, tag="wk_sb")
    wv_sb = persist.tile([128, KC * MYC], MMDT, tag="wv_sb")
    wpA = persist.tile([128, C], MMDT, tag="wpA")
    wpB = persist.tile([64, C], MMDT, tag="wpB")
    bqA = persist.tile([128, 1], F32, tag="bqA")
    bqB = persist.tile([64, 1], F32, tag="bqB")
    bkA = persist.tile([128, 1], F32, tag="bkA")
    bkB = persist.tile([64, 1], F32, tag="bkB")
    bv_row = persist.tile([1, MYC], MMDT, tag="bv_row")
    ones = persist.tile([1, 128], MMDT, tag="ones")
    shift_col = persist.tile([128, 1], F32, tag="shift_col")
    qTA = persist.tile([128, N], MMDT, tag="qTA")
    kTA = persist.tile([128, N], MMDT, tag="kTA")
    # head 2 k/q live duplicated on both partition halves (kj even/odd packing)
    qTB = persist.tile([128, N], MMDT, tag="qTB")
    kTB = persist.tile([128, N], MMDT, tag="kTB")
    v_sb = persist.tile([128, NTT * HPC * 65], MMDT, tag="v_sb")
    yTA = persist.tile([128, N], MMDT, tag="yTA")
    yTB = persist.tile([64, N], MMDT, tag="yTB")

    # ---- input DMAs: qk-proj operands first so phase 1 starts ASAP ----
    for kc in range(KC):
        nc.sync.dma_start(out=xT_sb[:, kc * N:(kc + 1) * N],
                          in_=xT[kc * 128:(kc + 1) * 128, :])
        nc.sync.dma_start(out=wq_sb[:, kc * MYC:(kc + 1) * MYC],
                          in_=wq[kc * 128:(kc + 1) * 128, :])
        nc.sync.dma_start(out=wk_sb[:, kc * MYC:(kc + 1) * MYC],
                          in_=wk[kc * 128:(kc + 1) * 128, :])
    nc.sync.dma_start(out=bqA, in_=bq[0:128, :])
    nc.sync.dma_start(out=bqB, in_=bq[128:MYC, :])
    nc.sync.dma_start(out=bkA, in_=bk[0:128, :])
    nc.sync.dma_start(out=bkB, in_=bk[128:MYC, :])
    for kc in range(KC):
        nc.sync.dma_start(out=wv_sb[:, kc * MYC:(kc + 1) * MYC],
                          in_=wv[kc * 128:(kc + 1) * 128, :])
    nc.sync.dma_start(out=bv_row, in_=bv)
    nc.sync.dma_start(out=wpA, in_=wp[0:128, :])
    nc.sync.dma_start(out=wpB, in_=wp[128:MYC, :])
    ones_f32 = persist.tile([128, 1], F32, tag="ones_f32")
    ones_row_f32 = persist.tile([1, 128], F32, tag="ones_row_f32")
    nc.vector.memset(ones_f32, 1.0)
    nc.vector.memset(ones_row_f32, 1.0)
    nc.vector.tensor_copy(out=ones, in_=ones_row_f32)
    nc.vector.memset(shift_col, EXP_SHIFT)

    # ---- phases 1+2: q/k/v projections (own PSUM pool, released after) ----
    with tc.tile_pool(name="ps_proj", bufs=2, space="PSUM") as ps_proj:
        for wsb, dstA, dstB, biasA, biasB in (
            (wq_sb, qTA, qTB, bqA, bqB),
            (wk_sb, kTA, kTB, bkA, bkB),
        ):
            for mg in range(2):
                M = 128 if mg == 0 else 64
                dst = dstA if mg == 0 else dstB
                bias = biasA if mg == 0 else biasB
                pss = [ps_proj.tile([M, QB], F32, tag="ps_qk", bufs=5,
                                    name=f"ps_qk{_i}")
                       for _i in range(N // QB)]
                for kc in range(KC):  # kc outer: overlap the xT load
                    for nt in range(N // QB):
                        nc.tensor.matmul(
                            pss[nt],
                            wsb[:, kc * MYC + mg * 128: kc * MYC + mg * 128 + M],
                            xT_sb[:, kc * N + nt * QB: kc * N + nt * QB + QB],
                            start=(kc == 0), stop=(kc == KC - 1),
                        )
                for nt in range(N // QB):
                    nc.vector.tensor_scalar(
                        out=dst[0:M, nt * QB:(nt + 1) * QB], in0=pss[nt],
                        scalar1=bias[0:M, :], scalar2=None, op0=OP.add,
                    )
        # duplicate head-2 k/q onto partitions 64..127 (cross-partition: DMA)
        nc.sync.dma_start(out=qTB[64:128, :], in_=qTB[0:64, :])
        nc.sync.dma_start(out=kTB[64:128, :], in_=kTB[0:64, :])

        for nt in range(NTT):
            ps = ps_proj.tile([128, MYC], F32, tag="ps_v")
            for kc in range(KC):
                nc.tensor.matmul(
                    ps,
                    xT_sb[:, kc * N + nt * 128: kc * N + nt * 128 + 128],
                    wv_sb[:, kc * MYC:(kc + 1) * MYC],
                    start=(kc == 0), stop=False,
                )
            nc.tensor.matmul(ps, ones[0:1, 0:128], bv_row,
                             start=False, stop=True)
            for h in range(HPC):
                base = (nt * HPC + h) * 65
                nc.vector.tensor_copy(out=v_sb[:, base:base + 64],
                                      in_=ps[:, h * 64:(h + 1) * 64])
                nc.vector.tensor_copy(out=v_sb[:, base + 64:base + 65],
                                      in_=ones_f32)

    phases = os.environ.get("K_PHASES", "1234")
    if "3" not in phases:
        for i, src_t in enumerate((qTA, kTA, qTB, v_sb)):
            dump = ostage.tile([128, C], F32, name=f"dump{i}")
            nc.vector.tensor_copy(out=dump, in_=src_t[:, 0:C])
            nc.sync.dma_start(out=out[i * 128:(i + 1) * 128, :], in_=dump)
        return

    # ---- phase 3: attention; unit = (head-pair, qi block of 512) ----
    def vh_ap(kj, h):
        base = (kj * HPC + h) * 65
        return v_sb[:, base:base + 65]

    dram_bc = pools["dram_bc"]

    def normalize(yt, ydst, q0):
        den = small.tile([1, QB], F32, tag="den")
        nc.vector.tensor_copy(out=den, in_=yt[64:65, :])
        rec = small.tile([1, QB], F32, tag="rec")
        nc.vector.reciprocal_approx_fast(rec, den)
        dr = dram_bc.tile([1, QB], F32)
        nc.sync.dma_start(out=dr, in_=rec)
        bc = small.tile([64, QB], F32, tag="bc_sb")
        nc.sync.dma_start(out=bc, in_=_bcast_parts(dr, 64))
        nc.vector.scalar_tensor_tensor(
            out=ydst[:, q0:q0 + QB], in0=yt[0:64, :], scalar=1.0, in1=bc,
            op0=OP.mult, op1=OP.mult,
        )

    def proj_block(ps_st, qq):
        # projection for qi tiles of block qq; psum carved from st-pool slots
        # (two [128,384] outs in the two banks of one [128,1024] slot)
        for qt in range(qq * 4, qq * 4 + 4):
            stt = ps_st.tile([128, 1024], F32, tag="st", name=f"pj{qt}")
            ob = ostage.tile([128, C], F32, name=f"ob{qt}")
            for nb in range(2):
                po = stt[:, nb * QB: nb * QB + 384]
                nc.tensor.matmul(po, yTA[:, qt * 128:(qt + 1) * 128],
                                 wpA[:, nb * 384:(nb + 1) * 384],
                                 start=True, stop=False)
                nc.tensor.matmul(po, yTB[0:64, qt * 128:(qt + 1) * 128],
                                 wpB[0:64, nb * 384:(nb + 1) * 384],
                                 start=False, stop=True)
                nc.vector.tensor_copy(out=ob[:, nb * 384:(nb + 1) * 384],
                                      in_=po)
            nc.sync.dma_start(out=out[qt * 128:(qt + 1) * 128, :], in_=ob)

    with tc.tile_pool(name="ps_st", bufs=2, space="PSUM") as ps_st, \
         tc.tile_pool(name="ps_yt", bufs=4, space="PSUM") as ps_yt:
        for qq in range(4):
            q0 = qq * QB

            # --- head 2, even/odd kj pairs on the PE array halves ---
            yt2 = ps_yt.tile([65, QB], F32, tag="yt")
            prev = None
            for kp in range(NTT // 2):
                kj0, kj1 = 2 * kp, 2 * kp + 1
                st = ps_st.tile([128, 1024], F32, tag="st")
                nc.tensor.matmul(st[:, 0:QB],
                                 kTB[0:64, kj0 * 128:(kj0 + 1) * 128],
                                 qTB[0:64, q0:q0 + QB], start=True, stop=True)
                nc.tensor.matmul(st[:, QB:1024],
                                 kTB[64:128, kj1 * 128:(kj1 + 1) * 128],
                                 qTB[64:128, q0:q0 + QB], start=True, stop=True)
                et = et_pool.tile([128, 1024], MMDT)
                nc.scalar.activation(et, st, AF.Exp, bias=shift_col[:, :])
                if prev is not None:
                    pet, pkp = prev
                    nc.tensor.matmul(yt2, vh_ap(2 * pkp, 2), pet[:, 0:QB],
                                     start=(pkp == 0), stop=False)
                    nc.tensor.matmul(yt2, vh_ap(2 * pkp + 1, 2),
                                     pet[:, QB:1024], start=False, stop=False)
                prev = (et, kp)
            pet, pkp = prev
            nc.tensor.matmul(yt2, vh_ap(2 * pkp, 2), pet[:, 0:QB],
                             start=(pkp == 0), stop=False)
            nc.tensor.matmul(yt2, vh_ap(2 * pkp + 1, 2), pet[:, QB:1024],
                             start=False, stop=True)
            normalize(yt2, yTB[0:64, :], q0)

            # --- heads 0+1, row-paired on the PE array ---
            yt0 = ps_yt.tile([65, QB], F32, tag="yt")
            yt1 = ps_yt.tile([65, QB], F32, tag="yt")
            prev = None
            for kj in range(NTT):
                st = ps_st.tile([128, 1024], F32, tag="st")
                nc.tensor.matmul(st[:, 0:QB],
                                 kTA[0:64, kj * 128:(kj + 1) * 128],
                                 qTA[0:64, q0:q0 + QB], start=True, stop=True)
                nc.tensor.matmul(st[:, QB:1024],
                                 kTA[64:128, kj * 128:(kj + 1) * 128],
                                 qTA[64:128, q0:q0 + QB], start=True, stop=True)
                et = et_pool.tile([128, 1024], MMDT)
                nc.scalar.activation(et, st, AF.Exp, bias=shift_col[:, :])
                if prev is not None:
                    pet, pkj = prev
                    nc.tensor.matmul(yt0, vh_ap(pkj, 0), pet[:, 0:QB],
                                     start=(pkj == 0), stop=False)
                    nc.tensor.matmul(yt1, vh_ap(pkj, 1), pet[:, QB:1024],
                                     start=(pkj == 0), stop=False)
                prev = (et, kj)
            pet, pkj = prev
            nc.tensor.matmul(yt0, vh_ap(pkj, 0), pet[:, 0:QB],
                             start=False, stop=True)
            nc.tensor.matmul(yt1, vh_ap(pkj, 1), pet[:, QB:1024],
                             start=False, stop=True)
            normalize(yt0, yTA[0:64, :], q0)
            normalize(yt1, yTA[64:128, :], q0)

            # projection for the PREVIOUS block overlaps this block's drain
            if os.environ.get("K_PROJ", "fused") == "fused":
                if qq > 0:
                    proj_block(ps_st, qq - 1)
        if os.environ.get("K_PROJ", "fused") == "fused":
            proj_block(ps_st, 3)
        else:
            for qq in range(4):
                proj_block(ps_st, qq)




def _build_program():
    nc = bacc.Bacc("TRN2", target_bir_lowering=False, debug=False,
                   num_devices=NCORES)
    aps = {
        "xT": nc.dram_tensor("xT", [C, N], MMDT, kind="ExternalInput").ap(),
        "wq": nc.dram_tensor("wq", [C, MYC], MMDT, kind="ExternalInput").ap(),
        "wk": nc.dram_tensor("wk", [C, MYC], MMDT, kind="ExternalInput").ap(),
        "wv": nc.dram_tensor("wv", [C, MYC], MMDT, kind="ExternalInput").ap(),
        "wp": nc.dram_tensor("wp", [MYC, C], MMDT, kind="ExternalInput").ap(),
        "bq": nc.dram_tensor("bq", [MYC, 1], F32, kind="ExternalInput").ap(),
        "bk": nc.dram_tensor("bk", [MYC, 1], F32, kind="ExternalInput").ap(),
        "bv": nc.dram_tensor("bv", [1, MYC], MMDT, kind="ExternalInput").ap(),
        "out": nc.dram_tensor("out", [N, C], F32, kind="ExternalOutput").ap(),
    }
    with tile.TileContext(nc) as tc:
        import contextlib
        with contextlib.ExitStack() as ctx:
            pools = {
                "persist": ctx.enter_context(tc.tile_pool(name="persist", bufs=1)),
                "et": ctx.enter_context(tc.tile_pool(name="et", bufs=3)),
                "small": ctx.enter_context(tc.tile_pool(name="small", bufs=2)),
                "ostage": ctx.enter_context(tc.tile_pool(name="ostage", bufs=2)),
                "dram_bc": ctx.enter_context(
                    tc.tile_pool(name="dram_bc", bufs=2, space="DRAM")),
            }
            _emit(nc, tc, pools, aps)
    nc.compile()
    return nc


_PROGRAM_CACHE = {}


def _get_program():
    if "nc" not in _PROGRAM_CACHE:
        _PROGRAM_CACHE["nc"] = _build_program()
    return _PROGRAM_CACHE["nc"]


def make_in_maps(x, Wq, bq, Wk, bk, Wv, bv, Wp, bp):
    scale = 1.0 / math.sqrt(DH)
    xTb = [np.ascontiguousarray(x[b].T) for b in range(B)]
    wire = mybir.dt.np(MMDT)
    in_maps = []
    for c in range(NCORES):
        b, hg = c // CPG, c % CPG
        cols = slice(hg * MYC, (hg + 1) * MYC)
        in_maps.append({
            "xT": xTb[b].astype(wire),
            "wq": (np.ascontiguousarray(Wq[:, cols]) * np.float32(scale)).astype(wire),
            "wk": np.ascontiguousarray(Wk[:, cols]).astype(wire),
            "wv": np.ascontiguousarray(Wv[:, cols]).astype(wire),
            "wp": np.ascontiguousarray(Wp[cols, :]).astype(wire),
            "bq": (bq[cols] * np.float32(scale)).reshape(MYC, 1).copy(),
            "bk": bk[cols].reshape(MYC, 1).copy(),
            "bv": bv[cols].reshape(1, MYC).astype(wire),
        })
    return in_maps


def assemble(results, bp):
    out = np.empty((B, N, C), np.float32)
    for b in range(B):
        acc = results[b * CPG]["out"].astype(np.float64)
        for c in range(b * CPG + 1, (b + 1) * CPG):
            acc = acc + results[c]["out"]
        out[b] = (acc + bp.astype(np.float64)).astype(np.float32)
    return out


def kernel(x, Wq, bq, Wk, bk, Wv, bv, Wp, bp, **extra_kwargs):
    x = np.asarray(x, np.float32)
    Wq = np.asarray(Wq, np.float32)
    Wk = np.asarray(Wk, np.float32)
    Wv = np.asarray(Wv, np.float32)
    Wp = np.asarray(Wp, np.float32)
    bq = np.asarray(bq, np.float32)
    bk = np.asarray(bk, np.float32)
    bv = np.asarray(bv, np.float32)
    bp = np.asarray(bp, np.float32)

    nc = _get_program()
    in_maps = make_in_maps(x, Wq, bq, Wk, bk, Wv, bv, Wp, bp)
    res = bass_utils.run_bass_kernel_spmd(nc, in_maps,
                                          core_ids=list(range(NCORES)))
    return assemble(res.results, bp)



# revision 8
# speedup vs baseline: 1.4430x; 1.2884x over previous
"""Multi-head attention (B=2, N=2048, C=768, H=12, DH=64) on 8 Trainium2 cores.

Sharding: data-parallel on batch (cores 0-3 -> b=0, cores 4-7 -> b=1),
tensor-parallel on heads within each group (3 heads/core: Wq/Wk/Wv column
slices, Wp row slices).  Each core emits its partial projection output
[N, C]; the host sums the 4 partials per batch and adds bp (cheaper than a
device collective at this size).

Per-core dataflow (feature-major, transpose-free, fp16 operands / fp32 psum):
  - host supplies xT = x[b].T  [C, N] in fp16
  - qT,kT [64, N] per head = W.T @ xT; each head's 64 dims then duplicated
    onto both PE-row halves (SBUF->SBUF DMA) so score matmuls pair
    even/odd kj tiles on disjoint PE row halves (co-execute)
  - v [N, 192] token-major from xT as lhsT, with a ones column per head
  - phase 3 is one continuous stream over (qq, head, kj): 192 score tiles
    STt [kj,qi] grouped 3 per [128,1536] psum tile; ONE exp ACT op per
    group (amortizes the ~370ns ACT fixed overhead); yT accumulation
    consumes ET one group behind (PE never waits on ACT)
  - yT_aug[65, qi] = [v_h | 1].T @ ET accumulated over kj; row 64 = denom
  - normalize: denom -> sbuf copy, reciprocal_approx_fast, DMA broadcast
    via DRAM, fused multiply on gpsimd (keeps DVE queue short)
  - out[qi, C] partial = yT (stationary) @ Wp rows, interleaved into the
    next block's stream, psum borrowed from the st pool
"""

import math

import numpy as np

import concourse.bacc as bacc
import concourse.bass as bass
import concourse.mybir as mybir
import concourse.tile as tile
from concourse import bass_utils

B, N, C, H, DH = 2, 2048, 768, 12, 64
NCORES = 8
CPG = 4                  # cores per batch group
HPC = H // CPG           # heads per core = 3
MYC = HPC * DH           # per-core feature width = 192
KC = C // 128            # contraction chunks = 6
NTT = N // 128           # token tiles = 16
QB = 512                 # qi block (psum bank width, fp32)
F32 = mybir.dt.float32
MMDT = mybir.dt.float16  # matmul operand dtype
AF = mybir.ActivationFunctionType
OP = mybir.AluOpType

EXP_SHIFT = -4.0         # exp(s + EXP_SHIFT); cancels between num and denom


def _bcast_parts(ap, nparts):
    """Partition-stride-0 broadcast view of a [1, F] AP (DMA source only)."""
    return bass.AP(tensor=ap.tensor, offset=ap.offset,
                   ap=[[0, nparts]] + [list(d) for d in ap.ap[1:]])


def _emit(nc, tc, pools, aps):
    xT, wq, wk, wv, wp, bq, bk, bv, out = (
        aps["xT"], aps["wq"], aps["wk"], aps["wv"], aps["wp"],
        aps["bq"], aps["bk"], aps["bv"], aps["out"],
    )
    persist = pools["persist"]
    et_pool = pools["et"]
    small = pools["small"]
    ostage = pools["ostage"]
    dram_bc = pools["dram_bc"]

    # ---- persistent SBUF tensors ----
    xT_sb = persist.tile([128, KC * N], MMDT, tag="xT_sb")
    wq_sb = persist.tile([128, KC * MYC], MMDT, tag="wq_sb")
    wk_sb = persist.tile([128, KC * MYC], MMDT, tag="wk_sb")
    wv_sb = persist.tile([128, KC * MYC], MMDT, tag="wv_sb")
    wpA = persist.tile([128, C], MMDT, tag="wpA")
    wpB = persist.tile([64, C], MMDT, tag="wpB")
    bqA = persist.tile([128, 1], F32, tag="bqA")
    bqB = persist.tile([64, 1], F32, tag="bqB")
    bkA = persist.tile([128, 1], F32, tag="bkA")
    bkB = persist.tile([64, 1], F32, tag="bkB")
    bv_row = persist.tile([1, MYC], MMDT, tag="bv_row")
    ones = persist.tile([1, 128], MMDT, tag="ones")
    shift_col = persist.tile([128, 1], F32, tag="shift_col")
    # compact projections (h0 on parts 0:64, h1 on 64:128; h2 separate)
    qTA = persist.tile([128, N], MMDT, tag="qTA")
    kTA = persist.tile([128, N], MMDT, tag="kTA")
    # per-head partition-duplicated k/q for even/odd kj pair packing
    qTD = [persist.tile([128, N], MMDT, tag=f"qTD{h}", name=f"qTD{h}")
           for h in range(HPC)]
    kTD = [persist.tile([128, N], MMDT, tag=f"kTD{h}", name=f"kTD{h}")
           for h in range(HPC)]
    v_sb = persist.tile([128, NTT * HPC * 65], MMDT, tag="v_sb")
    yTA = persist.tile([128, N], MMDT, tag="yTA")
    yTB = persist.tile([64, N], MMDT, tag="yTB")

    # ---- input DMAs: qk-proj operands first so phase 1 starts ASAP ----
    for kc in range(KC):
        nc.sync.dma_start(out=xT_sb[:, kc * N:(kc + 1) * N],
                          in_=xT[kc * 128:(kc + 1) * 128, :])
        nc.sync.dma_start(out=wq_sb[:, kc * MYC:(kc + 1) * MYC],
                          in_=wq[kc * 128:(kc + 1) * 128, :])
        nc.sync.dma_start(out=wk_sb[:, kc * MYC:(kc + 1) * MYC],
                          in_=wk[kc * 128:(kc + 1) * 128, :])
    nc.sync.dma_start(out=bqA, in_=bq[0:128, :])
    nc.sync.dma_start(out=bqB, in_=bq[128:MYC, :])
    nc.sync.dma_start(out=bkA, in_=bk[0:128, :])
    nc.sync.dma_start(out=bkB, in_=bk[128:MYC, :])
    for kc in range(KC):
        nc.sync.dma_start(out=wv_sb[:, kc * MYC:(kc + 1) * MYC],
                          in_=wv[kc * 128:(kc + 1) * 128, :])
    nc.sync.dma_start(out=bv_row, in_=bv)
    nc.sync.dma_start(out=wpA, in_=wp[0:128, :])
    nc.sync.dma_start(out=wpB, in_=wp[128:MYC, :])
    ones_f32 = persist.tile([128, 1], F32, tag="ones_f32")
    ones_row_f32 = persist.tile([1, 128], F32, tag="ones_row_f32")
    nc.vector.memset(ones_f32, 1.0)
    nc.vector.memset(ones_row_f32, 1.0)
    nc.vector.tensor_copy(out=ones, in_=ones_row_f32)
    nc.vector.memset(shift_col, EXP_SHIFT)

    # ---- phases 1+2: q/k/v projections (own PSUM pool, released after) ----
    with tc.tile_pool(name="ps_proj", bufs=2, space="PSUM") as ps_proj:
        for wsb, dstA, dstB, biasA, biasB in (
            (wq_sb, qTA, qTD[2], bqA, bqB),
            (wk_sb, kTA, kTD[2], bkA, bkB),
        ):
            for mg in range(2):
                M = 128 if mg == 0 else 64
                dst = dstA if mg == 0 else dstB
                bias = biasA if mg == 0 else biasB
                pss = [ps_proj.tile([M, QB], F32, tag="ps_qk", bufs=5,
                                    name=f"ps_qk{_i}")
                       for _i in range(N // QB)]
                for kc in range(KC):  # kc outer: overlap the xT load
                    for nt in range(N // QB):
                        nc.tensor.matmul(
                            pss[nt],
                            wsb[:, kc * MYC + mg * 128: kc * MYC + mg * 128 + M],
                            xT_sb[:, kc * N + nt * QB: kc * N + nt * QB + QB],
                            start=(kc == 0), stop=(kc == KC - 1),
                        )
                for nt in range(N // QB):
                    nc.vector.tensor_scalar(
                        out=dst[0:M, nt * QB:(nt + 1) * QB], in0=pss[nt],
                        scalar1=bias[0:M, :], scalar2=None, op0=OP.add,
                    )
        # duplicate each head's 64 dims onto both partition halves (DMA)
        nc.sync.dma_start(out=qTD[2][64:128, :], in_=qTD[2][0:64, :])
        nc.sync.dma_start(out=kTD[2][64:128, :], in_=kTD[2][0:64, :])
        nc.sync.dma_start(out=qTD[0][0:64, :], in_=qTA[0:64, :])
        nc.sync.dma_start(out=qTD[0][64:128, :], in_=qTA[0:64, :])
        nc.sync.dma_start(out=qTD[1][0:64, :], in_=qTA[64:128, :])
        nc.sync.dma_start(out=qTD[1][64:128, :], in_=qTA[64:128, :])
        nc.sync.dma_start(out=kTD[0][0:64, :], in_=kTA[0:64, :])
        nc.sync.dma_start(out=kTD[0][64:128, :], in_=kTA[0:64, :])
        nc.sync.dma_start(out=kTD[1][0:64, :], in_=kTA[64:128, :])
        nc.sync.dma_start(out=kTD[1][64:128, :], in_=kTA[64:128, :])

        for nt in range(NTT):
            ps = ps_proj.tile([128, MYC], F32, tag="ps_v")
            for kc in range(KC):
                nc.tensor.matmul(
                    ps,
                    xT_sb[:, kc * N + nt * 128: kc * N + nt * 128 + 128],
                    wv_sb[:, kc * MYC:(kc + 1) * MYC],
                    start=(kc == 0), stop=False,
                )
            nc.tensor.matmul(ps, ones[0:1, 0:128], bv_row,
                             start=False, stop=True)
            for h in range(HPC):
                base = (nt * HPC + h) * 65
                nc.vector.tensor_copy(out=v_sb[:, base:base + 64],
                                      in_=ps[:, h * 64:(h + 1) * 64])
                nc.vector.tensor_copy(out=v_sb[:, base + 64:base + 65],
                                      in_=ones_f32)

    # ---- phase 3: continuous stream over (qq, head, kj) ----
    def vh_ap(kj, h):
        base = (kj * HPC + h) * 65
        return v_sb[:, base:base + 65]

    # normalize: denom row -> sbuf, fast reciprocal, broadcast via DRAM,
    # fused multiply on DVE (gpsimd cannot read PSUM)
    def normalize(yt, h, qq):
        q0 = qq * QB
        ydst = yTA[0:64, :] if h == 0 else (
            yTA[64:128, :] if h == 1 else yTB[0:64, :])
        den = small.tile([1, QB], F32, tag="den")
        nc.vector.tensor_copy(out=den, in_=yt[64:65, :])
        rec = small.tile([1, QB], F32, tag="rec")
        nc.vector.reciprocal_approx_fast(rec, den)
        dr = dram_bc.tile([1, QB], F32, tag="dr", name=f"dr{h}_{qq}")
        nc.sync.dma_start(out=dr, in_=rec)
        bc = small.tile([64, QB], F32, tag="bc_sb")
        nc.sync.dma_start(out=bc, in_=_bcast_parts(dr, 64))
        nc.vector.scalar_tensor_tensor(
            out=ydst[:, q0:q0 + QB], in0=yt[0:64, :], scalar=1.0, in1=bc,
            op0=OP.mult, op1=OP.mult,
        )

    def proj_qt(ps_st, qt):
        # one output row-tile [128, C]; psum borrowed from the st pool
        stt = ps_st.tile([128, 3 * QB], F32, tag="st", name=f"pj{qt}")
        ob = ostage.tile([128, C], F32, name=f"ob{qt}")
        for nb in range(2):
            po = stt[:, nb * QB: nb * QB + 384]
            nc.tensor.matmul(po, yTA[:, qt * 128:(qt + 1) * 128],
                             wpA[:, nb * 384:(nb + 1) * 384],
                             start=True, stop=False)
            nc.tensor.matmul(po, yTB[0:64, qt * 128:(qt + 1) * 128],
                             wpB[0:64, nb * 384:(nb + 1) * 384],
                             start=False, stop=True)
            nc.vector.tensor_copy(out=ob[:, nb * 384:(nb + 1) * 384],
                                  in_=po)
        nc.sync.dma_start(out=out[qt * 128:(qt + 1) * 128, :], in_=ob)

    stream = [(qq, h, kj)
              for qq in range(4) for h in range(HPC) for kj in range(NTT)]
    NG = len(stream) // 3  # 64 groups of 3 score tiles

    # proj for block qq interleaved into block qq+1's stream: qt j of qq
    # at stream entry (qq+1)*48 + 18 + 6j -> group index
    proj_at = {}
    for qq in range(3):
        for j in range(4):
            g_ins = ((qq + 1) * 48 + 18 + 6 * j) // 3
            proj_at.setdefault(g_ins, []).append(qq * 4 + j)

    with tc.tile_pool(name="ps_st", bufs=2, space="PSUM") as ps_st, \
         tc.tile_pool(name="ps_yt", bufs=2, space="PSUM") as ps_yt:
        yt_cur = {}
        prev = None
        for g in range(NG + 1):
            if g < NG:
                entries = [stream[3 * g + j] for j in range(3)]
                st = ps_st.tile([128, 3 * QB], F32, tag="st", name=f"st{g}")
                for j, (qq, h, kj) in enumerate(entries):
                    lo = 0 if kj % 2 == 0 else 64
                    nc.tensor.matmul(
                        st[:, j * QB:(j + 1) * QB],
                        kTD[h][lo:lo + 64, kj * 128:(kj + 1) * 128],
                        qTD[h][lo:lo + 64, qq * QB:(qq + 1) * QB],
                        start=True, stop=True,
                    )
                et = et_pool.tile([128, 3 * QB], MMDT, tag="et", name=f"et{g}")
                nc.scalar.activation(et, st, AF.Exp, bias=shift_col[:, :])
            if prev is not None:
                pet, pentries = prev
                for j, (qq, h, kj) in enumerate(pentries):
                    if kj == 0:
                        yt_cur[(qq, h)] = ps_yt.tile([65, QB], F32, tag="yt",
                                                     name=f"yt{qq}_{h}")
                    nc.tensor.matmul(yt_cur[(qq, h)], vh_ap(kj, h),
                                     pet[:, j * QB:(j + 1) * QB],
                                     start=(kj == 0), stop=(kj == NTT - 1))
                    if kj == NTT - 1:
                        normalize(yt_cur.pop((qq, h)), h, qq)
            prev = (et, entries) if g < NG else None
            for qt in proj_at.get(g + 1, []):
                proj_qt(ps_st, qt)
        for qt in range(12, 16):
            proj_qt(ps_st, qt)


def _build_program():
    nc = bacc.Bacc("TRN2", target_bir_lowering=False, debug=False,
                   num_devices=NCORES)
    aps = {
        "xT": nc.dram_tensor("xT", [C, N], MMDT, kind="ExternalInput").ap(),
        "wq": nc.dram_tensor("wq", [C, MYC], MMDT, kind="ExternalInput").ap(),
        "wk": nc.dram_tensor("wk", [C, MYC], MMDT, kind="ExternalInput").ap(),
        "wv": nc.dram_tensor("wv", [C, MYC], MMDT, kind="ExternalInput").ap(),
        "wp": nc.dram_tensor("wp", [MYC, C], MMDT, kind="ExternalInput").ap(),
        "bq": nc.dram_tensor("bq", [MYC, 1], F32, kind="ExternalInput").ap(),
        "bk": nc.dram_tensor("bk", [MYC, 1], F32, kind="ExternalInput").ap(),
        "bv": nc.dram_tensor("bv", [1, MYC], MMDT, kind="ExternalInput").ap(),
        "out": nc.dram_tensor("out", [N, C], F32, kind="ExternalOutput").ap(),
    }
    with tile.TileContext(nc) as tc:
        import contextlib
        with contextlib.ExitStack() as ctx:
            pools = {
                "persist": ctx.enter_context(tc.tile_pool(name="persist", bufs=1)),
                "et": ctx.enter_context(tc.tile_pool(name="et", bufs=3)),
                "small": ctx.enter_context(tc.tile_pool(name="small", bufs=3)),
                "ostage": ctx.enter_context(tc.tile_pool(name="ostage", bufs=2)),
                "dram_bc": ctx.enter_context(
                    tc.tile_pool(name="dram_bc", bufs=2, space="DRAM")),
            }
            _emit(nc, tc, pools, aps)
    nc.compile()
    return nc


_PROGRAM_CACHE = {}


def _get_program():
    if "nc" not in _PROGRAM_CACHE:
        _PROGRAM_CACHE["nc"] = _build_program()
    return _PROGRAM_CACHE["nc"]


def make_in_maps(x, Wq, bq, Wk, bk, Wv, bv, Wp, bp):
    scale = 1.0 / math.sqrt(DH)
    xTb = [np.ascontiguousarray(x[b].T) for b in range(B)]
    wire = mybir.dt.np(MMDT)
    in_maps = []
    for c in range(NCORES):
        b, hg = c // CPG, c % CPG
        cols = slice(hg * MYC, (hg + 1) * MYC)
        in_maps.append({
            "xT": xTb[b].astype(wire),
            "wq": (np.ascontiguousarray(Wq[:, cols]) * np.float32(scale)).astype(wire),
            "wk": np.ascontiguousarray(Wk[:, cols]).astype(wire),
            "wv": np.ascontiguousarray(Wv[:, cols]).astype(wire),
            "wp": np.ascontiguousarray(Wp[cols, :]).astype(wire),
            "bq": (bq[cols] * np.float32(scale)).reshape(MYC, 1).copy(),
            "bk": bk[cols].reshape(MYC, 1).copy(),
            "bv": bv[cols].reshape(1, MYC).astype(wire),
        })
    return in_maps


def assemble(results, bp):
    out = np.empty((B, N, C), np.float32)
    for b in range(B):
        acc = results[b * CPG]["out"].astype(np.float64)
        for c in range(b * CPG + 1, (b + 1) * CPG):
            acc = acc + results[c]["out"]
        out[b] = (acc + bp.astype(np.float64)).astype(np.float32)
    return out


def kernel(x, Wq, bq, Wk, bk, Wv, bv, Wp, bp, **extra_kwargs):
    x = np.asarray(x, np.float32)
    Wq = np.asarray(Wq, np.float32)
    Wk = np.asarray(Wk, np.float32)
    Wv = np.asarray(Wv, np.float32)
    Wp = np.asarray(Wp, np.float32)
    bq = np.asarray(bq, np.float32)
    bk = np.asarray(bk, np.float32)
    bv = np.asarray(bv, np.float32)
    bp = np.asarray(bp, np.float32)

    nc = _get_program()
    in_maps = make_in_maps(x, Wq, bq, Wk, bk, Wv, bv, Wp, bp)
    res = bass_utils.run_bass_kernel_spmd(nc, in_maps,
                                          core_ids=list(range(NCORES)))
    return assemble(res.results, bp)


# revision 13
# speedup vs baseline: 1.4540x; 1.0076x over previous
"""Multi-head attention (B=2, N=2048, C=768, H=12, DH=64) on 8 Trainium2 cores.

Sharding: data-parallel on batch (cores 0-3 -> b=0, cores 4-7 -> b=1),
tensor-parallel on heads within each group (3 heads/core: Wq/Wk/Wv column
slices, Wp row slices).  Each core emits its partial projection output
[N, C]; the host sums the 4 partials per batch and adds bp (cheaper than a
device collective at this size).

Per-core dataflow (feature-major, transpose-free, fp16 operands / fp32 psum):
  - host supplies xT = x[b].T  [C, N] in fp16
  - qT,kT [64, N] per head = W.T @ xT; each head's 64 dims then duplicated
    onto both PE-row halves (SBUF->SBUF DMA) so score matmuls pair
    even/odd kj tiles on disjoint PE row halves (co-execute)
  - v [N, 192] token-major from xT as lhsT, with a ones column per head
  - phase 3 is one continuous stream over (qq, head, kj): 192 score tiles
    STt [kj,qi] grouped 3 per [128,1536] psum tile; ONE exp ACT op per
    group (amortizes the ~370ns ACT fixed overhead); yT accumulation
    consumes ET one group behind (PE never waits on ACT)
  - yT_aug[65, qi] = [v_h | 1].T @ ET accumulated over kj; row 64 = denom
  - normalize: denom -> sbuf copy, reciprocal_approx_fast, DMA broadcast
    via DRAM, fused multiply on gpsimd (keeps DVE queue short)
  - out[qi, C] partial = yT (stationary) @ Wp rows, interleaved into the
    next block's stream, psum borrowed from the st pool
"""

import math

import numpy as np

import concourse.bacc as bacc
import concourse.bass as bass
import concourse.mybir as mybir
import concourse.tile as tile
from concourse import bass_utils

B, N, C, H, DH = 2, 2048, 768, 12, 64
NCORES = 8
CPG = 4                  # cores per batch group
HPC = H // CPG           # heads per core = 3
MYC = HPC * DH           # per-core feature width = 192
KC = C // 128            # contraction chunks = 6
NTT = N // 128           # token tiles = 16
QB = 512                 # qi block (psum bank width, fp32)
F32 = mybir.dt.float32
MMDT = mybir.dt.float16  # matmul operand dtype
AF = mybir.ActivationFunctionType
OP = mybir.AluOpType

EXP_SHIFT = -4.0         # exp(s + EXP_SHIFT); cancels between num and denom


def _bcast_parts(ap, nparts):
    """Partition-stride-0 broadcast view of a [1, F] AP (DMA source only)."""
    return bass.AP(tensor=ap.tensor, offset=ap.offset,
                   ap=[[0, nparts]] + [list(d) for d in ap.ap[1:]])


def _emit(nc, tc, pools, aps):
    xT, wq, wk, wv, wp, bq, bk, bv, out = (
        aps["xT"], aps["wq"], aps["wk"], aps["wv"], aps["wp"],
        aps["bq"], aps["bk"], aps["bv"], aps["out"],
    )
    persist = pools["persist"]
    et_pool = pools["et"]
    small = pools["small"]
    ostage = pools["ostage"]
    dram_bc = pools["dram_bc"]

    # ---- persistent SBUF tensors ----
    xT_sb = persist.tile([128, KC * N], MMDT, tag="xT_sb")
    wq_sb = persist.tile([128, KC * MYC], MMDT, tag="wq_sb")
    wk_sb = persist.tile([128, KC * MYC], MMDT, tag="wk_sb")
    wv_sb = persist.tile([128, KC * MYC], MMDT, tag="wv_sb")
    wpA = persist.tile([128, C], MMDT, tag="wpA")
    wpB = persist.tile([64, C], MMDT, tag="wpB")
    bqA = persist.tile([128, 1], F32, tag="bqA")
    bqB = persist.tile([64, 1], F32, tag="bqB")
    bkA = persist.tile([128, 1], F32, tag="bkA")
    bkB = persist.tile([64, 1], F32, tag="bkB")
    bv_row = persist.tile([1, MYC], MMDT, tag="bv_row")
    ones = persist.tile([1, 128], MMDT, tag="ones")
    shift_col = persist.tile([128, 1], F32, tag="shift_col")
    # compact projections (h0 on parts 0:64, h1 on 64:128; h2 separate)
    qTA = persist.tile([128, N], MMDT, tag="qTA")
    kTA = persist.tile([128, N], MMDT, tag="kTA")
    # per-head partition-duplicated k/q for even/odd kj pair packing
    qTD = [persist.tile([128, N], MMDT, tag=f"qTD{h}", name=f"qTD{h}")
           for h in range(HPC)]
    kTD = [persist.tile([128, N], MMDT, tag=f"kTD{h}", name=f"kTD{h}")
           for h in range(HPC)]
    v_sb = persist.tile([128, NTT * HPC * 65], MMDT, tag="v_sb")
    yTA = persist.tile([128, N], MMDT, tag="yTA")
    yTB = persist.tile([64, N], MMDT, tag="yTB")

    # ---- input DMAs: qk-proj operands first so phase 1 starts ASAP ----
    for kc in range(KC):
        nc.sync.dma_start(out=xT_sb[:, kc * N:(kc + 1) * N],
                          in_=xT[kc * 128:(kc + 1) * 128, :])
        nc.sync.dma_start(out=wq_sb[:, kc * MYC:(kc + 1) * MYC],
                          in_=wq[kc * 128:(kc + 1) * 128, :])
        nc.sync.dma_start(out=wk_sb[:, kc * MYC:(kc + 1) * MYC],
                          in_=wk[kc * 128:(kc + 1) * 128, :])
    nc.sync.dma_start(out=bqA, in_=bq[0:128, :])
    nc.sync.dma_start(out=bqB, in_=bq[128:MYC, :])
    nc.sync.dma_start(out=bkA, in_=bk[0:128, :])
    nc.sync.dma_start(out=bkB, in_=bk[128:MYC, :])
    for kc in range(KC):
        nc.sync.dma_start(out=wv_sb[:, kc * MYC:(kc + 1) * MYC],
                          in_=wv[kc * 128:(kc + 1) * 128, :])
    nc.sync.dma_start(out=bv_row, in_=bv)
    nc.sync.dma_start(out=wpA, in_=wp[0:128, :])
    nc.sync.dma_start(out=wpB, in_=wp[128:MYC, :])
    ones_f32 = persist.tile([128, 1], F32, tag="ones_f32")
    ones_row_f32 = persist.tile([1, 128], F32, tag="ones_row_f32")
    nc.vector.memset(ones_f32, 1.0)
    nc.vector.memset(ones_row_f32, 1.0)
    nc.vector.tensor_copy(out=ones, in_=ones_row_f32)
    nc.vector.memset(shift_col, EXP_SHIFT)

    # ---- phases 1+2: q/k/v projections (own PSUM pool, released after) ----
    with tc.tile_pool(name="ps_proj", bufs=2, space="PSUM") as ps_proj:
        for wsb, dstA, dstB, biasA, biasB in (
            (wq_sb, qTA, qTD[2], bqA, bqB),
            (wk_sb, kTA, kTD[2], bkA, bkB),
        ):
            # both mg passes interleaved per kc chunk so matmul consumption
            # (~1.7us/chunk) stays behind the xT DMA supply (~1.5us/chunk)
            pssA = [ps_proj.tile([128, QB], F32, tag="ps_qkA", bufs=4,
                                 name=f"ps_qkA{_i}")
                    for _i in range(N // QB)]
            pssB = [ps_proj.tile([64, QB], F32, tag="ps_qkB", bufs=4,
                                 name=f"ps_qkB{_i}")
                    for _i in range(N // QB)]
            for kc in range(KC):
                for nt in range(N // QB):
                    nc.tensor.matmul(
                        pssA[nt],
                        wsb[:, kc * MYC: kc * MYC + 128],
                        xT_sb[:, kc * N + nt * QB: kc * N + nt * QB + QB],
                        start=(kc == 0), stop=(kc == KC - 1),
                    )
                for nt in range(N // QB):
                    nc.tensor.matmul(
                        pssB[nt],
                        wsb[:, kc * MYC + 128: kc * MYC + 192],
                        xT_sb[:, kc * N + nt * QB: kc * N + nt * QB + QB],
                        start=(kc == 0), stop=(kc == KC - 1),
                    )
            for nt in range(N // QB):
                nc.vector.tensor_scalar(
                    out=dstA[:, nt * QB:(nt + 1) * QB], in0=pssA[nt],
                    scalar1=biasA, scalar2=None, op0=OP.add,
                )
                nc.vector.tensor_scalar(
                    out=dstB[0:64, nt * QB:(nt + 1) * QB], in0=pssB[nt],
                    scalar1=biasB, scalar2=None, op0=OP.add,
                )
        # duplicate each head's 64 dims onto both partition halves (DMA)
        nc.sync.dma_start(out=qTD[2][64:128, :], in_=qTD[2][0:64, :])
        nc.sync.dma_start(out=kTD[2][64:128, :], in_=kTD[2][0:64, :])
        nc.sync.dma_start(out=qTD[0][0:64, :], in_=qTA[0:64, :])
        nc.sync.dma_start(out=qTD[0][64:128, :], in_=qTA[0:64, :])
        nc.sync.dma_start(out=qTD[1][0:64, :], in_=qTA[64:128, :])
        nc.sync.dma_start(out=qTD[1][64:128, :], in_=qTA[64:128, :])
        nc.sync.dma_start(out=kTD[0][0:64, :], in_=kTA[0:64, :])
        nc.sync.dma_start(out=kTD[0][64:128, :], in_=kTA[0:64, :])
        nc.sync.dma_start(out=kTD[1][0:64, :], in_=kTA[64:128, :])
        nc.sync.dma_start(out=kTD[1][64:128, :], in_=kTA[64:128, :])

        for nt in range(NTT):
            # reuse the ps_qkB tag's slots (freed by the bias adds)
            ps = ps_proj.tile([128, MYC], F32, tag="ps_qkB", bufs=4,
                              name=f"ps_v{nt}")
            for kc in range(KC):
                nc.tensor.matmul(
                    ps,
                    xT_sb[:, kc * N + nt * 128: kc * N + nt * 128 + 128],
                    wv_sb[:, kc * MYC:(kc + 1) * MYC],
                    start=(kc == 0), stop=False,
                )
            nc.tensor.matmul(ps, ones[0:1, 0:128], bv_row,
                             start=False, stop=True)
            for h in range(HPC):
                base = (nt * HPC + h) * 65
                nc.vector.tensor_copy(out=v_sb[:, base:base + 64],
                                      in_=ps[:, h * 64:(h + 1) * 64])
                nc.vector.tensor_copy(out=v_sb[:, base + 64:base + 65],
                                      in_=ones_f32)

    # ---- phase 3: continuous stream over (qq, head, kj) ----
    def vh_ap(kj, h):
        base = (kj * HPC + h) * 65
        return v_sb[:, base:base + 65]

    # normalize phase 1: denom row -> sbuf, fast reciprocal, launch the
    # DRAM-broadcast DMA.  The fused multiply (phase 2) is DEFERRED two
    # groups so its bc-DMA wait never head-of-line-blocks the DVE queue.
    def norm_start(yt, h, qq):
        den = small.tile([1, QB], F32, tag="den")
        nc.vector.tensor_copy(out=den, in_=yt[64:65, :])
        rec = small.tile([1, QB], F32, tag="rec")
        nc.vector.reciprocal_approx_fast(rec, den)
        dr = dram_bc.tile([1, QB], F32, tag="dr", name=f"dr{h}_{qq}")
        nc.sync.dma_start(out=dr, in_=rec)
        bc = small.tile([64, QB], F32, tag="bc_sb")
        nc.sync.dma_start(out=bc, in_=_bcast_parts(dr, 64))
        return (yt, bc, h, qq)

    def norm_finish(state):
        yt, bc, h, qq = state
        q0 = qq * QB
        ydst = yTA[0:64, :] if h == 0 else (
            yTA[64:128, :] if h == 1 else yTB[0:64, :])
        nc.vector.scalar_tensor_tensor(
            out=ydst[:, q0:q0 + QB], in0=yt[0:64, :], scalar=1.0, in1=bc,
            op0=OP.mult, op1=OP.mult,
        )

    def proj_full(ps_st, qt):
        # one output row-tile [128, C]; psum borrowed from the st pool
        stt = ps_st.tile([128, 3 * QB], F32, tag="st", name=f"pj{qt}")
        ob = ostage.tile([128, C], F32, name=f"ob{qt}")
        for nb in range(2):
            po = stt[:, nb * QB: nb * QB + 384]
            nc.tensor.matmul(po, yTA[:, qt * 128:(qt + 1) * 128],
                             wpA[:, nb * 384:(nb + 1) * 384],
                             start=True, stop=False)
            nc.tensor.matmul(po, yTB[0:64, qt * 128:(qt + 1) * 128],
                             wpB[0:64, nb * 384:(nb + 1) * 384],
                             start=False, stop=True)
            nc.vector.tensor_copy(out=ob[:, nb * 384:(nb + 1) * 384],
                                  in_=po)
        nc.sync.dma_start(out=out[qt * 128:(qt + 1) * 128, :], in_=ob)

    stream = [(qq, h, kj)
              for qq in range(4) for h in range(HPC) for kj in range(NTT)]
    NG = len(stream) // 3  # 64 groups of 3 score tiles

    # proj for block qq interleaved into block qq+1's stream, at group
    # offsets chosen to avoid the normalize seams (~+6 and ~+11.3)
    proj_at = {}
    for qq in range(3):
        for j, goff in enumerate((9, 10, 14, 15)):
            proj_at.setdefault((qq + 1) * 16 + goff, []).append(qq * 4 + j)

    with tc.tile_pool(name="ps_st", bufs=2, space="PSUM") as ps_st, \
         tc.tile_pool(name="ps_yt", bufs=2, space="PSUM") as ps_yt:
        yt_cur = {}
        pending = []  # (due_group, norm state)
        prev = None
        pjAB = []
        for g in range(NG + 1):
            if g < NG:
                entries = [stream[3 * g + j] for j in range(3)]
                st = ps_st.tile([128, 3 * QB], F32, tag="st", name=f"st{g}")
                for j, (qq, h, kj) in enumerate(entries):
                    lo = 0 if kj % 2 == 0 else 64
                    nc.tensor.matmul(
                        st[:, j * QB:(j + 1) * QB],
                        kTD[h][lo:lo + 64, kj * 128:(kj + 1) * 128],
                        qTD[h][lo:lo + 64, qq * QB:(qq + 1) * QB],
                        start=True, stop=True,
                    )
                et = et_pool.tile([128, 3 * QB], MMDT, tag="et", name=f"et{g}")
                nc.scalar.activation(et, st, AF.Exp, bias=shift_col[:, :])
            if prev is not None:
                pet, pentries = prev
                for j, (qq, h, kj) in enumerate(pentries):
                    if kj == 0:
                        yt_cur[(qq, h)] = ps_yt.tile([65, QB], F32, tag="yt",
                                                     name=f"yt{qq}_{h}")
                    nc.tensor.matmul(yt_cur[(qq, h)], vh_ap(kj, h),
                                     pet[:, j * QB:(j + 1) * QB],
                                     start=(kj == 0), stop=(kj == NTT - 1))
                    if kj == NTT - 1:
                        pending.append(
                            (g + 2, norm_start(yt_cur.pop((qq, h)), h, qq)))
            prev = (et, entries) if g < NG else None
            while pending and pending[0][0] <= g:
                norm_finish(pending.pop(0)[1])
            for qt in proj_at.get(g + 1, []):
                proj_full(ps_st, qt)
            if g == NG - 1:
                # final block's proj: h0+h1 contribution early (their
                # normalizes are done); h2 accumulated after the last one.
                # 8 bank-aligned po regions: 3+3 in the two st-pool slots,
                # 2 borrowed from the (now draining) yt pool.
                stA = ps_st.tile([128, 3 * QB], F32, tag="st", name="pjtA")
                stB = ps_st.tile([128, 3 * QB], F32, tag="st", name="pjtB")
                poY = [ps_yt.tile([128, 384], F32, tag="yt", name=f"poY{_i}")
                       for _i in range(2)]
                for p in range(8):
                    qt, nb = 12 + p // 2, p % 2
                    if p < 3:
                        po = stA[:, p * QB: p * QB + 384]
                    elif p < 6:
                        po = stB[:, (p - 3) * QB: (p - 3) * QB + 384]
                    else:
                        po = poY[p - 6][:, 0:384]
                    pjAB.append(po)
                    nc.tensor.matmul(po, yTA[:, qt * 128:(qt + 1) * 128],
                                     wpA[:, nb * 384:(nb + 1) * 384],
                                     start=True, stop=False)
        while pending:
            norm_finish(pending.pop(0)[1])
        for qx in range(4):
            qt = 12 + qx
            ob = ostage.tile([128, C], F32, name=f"ob{qt}")
            for nb in range(2):
                po = pjAB[qx * 2 + nb]
                nc.tensor.matmul(po, yTB[0:64, qt * 128:(qt + 1) * 128],
                                 wpB[0:64, nb * 384:(nb + 1) * 384],
                                 start=False, stop=True)
                nc.vector.tensor_copy(out=ob[:, nb * 384:(nb + 1) * 384],
                                      in_=po)
            nc.sync.dma_start(out=out[qt * 128:(qt + 1) * 128, :], in_=ob)


def _build_program():
    nc = bacc.Bacc("TRN2", target_bir_lowering=False, debug=False,
                   num_devices=NCORES)
    aps = {
        "xT": nc.dram_tensor("xT", [C, N], MMDT, kind="ExternalInput").ap(),
        "wq": nc.dram_tensor("wq", [C, MYC], MMDT, kind="ExternalInput").ap(),
        "wk": nc.dram_tensor("wk", [C, MYC], MMDT, kind="ExternalInput").ap(),
        "wv": nc.dram_tensor("wv", [C, MYC], MMDT, kind="ExternalInput").ap(),
        "wp": nc.dram_tensor("wp", [MYC, C], MMDT, kind="ExternalInput").ap(),
        "bq": nc.dram_tensor("bq", [MYC, 1], F32, kind="ExternalInput").ap(),
        "bk": nc.dram_tensor("bk", [MYC, 1], F32, kind="ExternalInput").ap(),
        "bv": nc.dram_tensor("bv", [1, MYC], MMDT, kind="ExternalInput").ap(),
        "out": nc.dram_tensor("out", [N, C], F32, kind="ExternalOutput").ap(),
    }
    with tile.TileContext(nc) as tc:
        import contextlib
        with contextlib.ExitStack() as ctx:
            pools = {
                "persist": ctx.enter_context(tc.tile_pool(name="persist", bufs=1)),
                "et": ctx.enter_context(tc.tile_pool(name="et", bufs=3)),
                "small": ctx.enter_context(tc.tile_pool(name="small", bufs=3)),
                "ostage": ctx.enter_context(tc.tile_pool(name="ostage", bufs=2)),
                "dram_bc": ctx.enter_context(
                    tc.tile_pool(name="dram_bc", bufs=2, space="DRAM")),
            }
            _emit(nc, tc, pools, aps)
    nc.compile()
    return nc


_PROGRAM_CACHE = {}


def _get_program():
    if "nc" not in _PROGRAM_CACHE:
        _PROGRAM_CACHE["nc"] = _build_program()
    return _PROGRAM_CACHE["nc"]


def make_in_maps(x, Wq, bq, Wk, bk, Wv, bv, Wp, bp):
    scale = 1.0 / math.sqrt(DH)
    xTb = [np.ascontiguousarray(x[b].T) for b in range(B)]
    wire = mybir.dt.np(MMDT)
    in_maps = []
    for c in range(NCORES):
        b, hg = c // CPG, c % CPG
        cols = slice(hg * MYC, (hg + 1) * MYC)
        in_maps.append({
            "xT": xTb[b].astype(wire),
            "wq": (np.ascontiguousarray(Wq[:, cols]) * np.float32(scale)).astype(wire),
            "wk": np.ascontiguousarray(Wk[:, cols]).astype(wire),
            "wv": np.ascontiguousarray(Wv[:, cols]).astype(wire),
            "wp": np.ascontiguousarray(Wp[cols, :]).astype(wire),
            "bq": (bq[cols] * np.float32(scale)).reshape(MYC, 1).copy(),
            "bk": bk[cols].reshape(MYC, 1).copy(),
            "bv": bv[cols].reshape(1, MYC).astype(wire),
        })
    return in_maps


def assemble(results, bp):
    out = np.empty((B, N, C), np.float32)
    for b in range(B):
        acc = results[b * CPG]["out"].astype(np.float64)
        for c in range(b * CPG + 1, (b + 1) * CPG):
            acc = acc + results[c]["out"]
        out[b] = (acc + bp.astype(np.float64)).astype(np.float32)
    return out


def kernel(x, Wq, bq, Wk, bk, Wv, bv, Wp, bp, **extra_kwargs):
    x = np.asarray(x, np.float32)
    Wq = np.asarray(Wq, np.float32)
    Wk = np.asarray(Wk, np.float32)
    Wv = np.asarray(Wv, np.float32)
    Wp = np.asarray(Wp, np.float32)
    bq = np.asarray(bq, np.float32)
    bk = np.asarray(bk, np.float32)
    bv = np.asarray(bv, np.float32)
    bp = np.asarray(bp, np.float32)

    nc = _get_program()
    in_maps = make_in_maps(x, Wq, bq, Wk, bk, Wv, bv, Wp, bp)
    res = bass_utils.run_bass_kernel_spmd(nc, in_maps,
                                          core_ids=list(range(NCORES)))
    return assemble(res.results, bp)


# revision 20
# speedup vs baseline: 1.5340x; 1.0550x over previous
"""Multi-head attention (B=2, N=2048, C=768, H=12, DH=64) on 8 Trainium2 cores.

Sharding: data-parallel on batch (cores 0-3 -> b=0, cores 4-7 -> b=1),
tensor-parallel on heads within each group (3 heads/core: Wq/Wk/Wv column
slices, Wp row slices).  Each core emits its partial projection output
[N, C]; the host sums the 4 partials per batch and adds bp (cheaper than a
device collective at this size).

Per-core dataflow (feature-major, transpose-free, fp16 operands / fp32 psum):
  - host supplies xT = x[b].T  [C, N] in fp16; h2's q and k weight columns
    are host-packed into one [C,128] tensor so all qk-proj matmuls are M=128
  - qT,kT [64, N] per head = W.T @ xT; each head's 64 dims then duplicated
    onto both PE-row halves (SBUF->SBUF DMA) so score matmuls pair
    even/odd kj tiles on disjoint PE row halves (co-execute)
  - v [N, 192] token-major from xT as lhsT, with a ones column per head;
    v psum borrowed from the yt pool so the score stream's first groups
    (ST + exp) prefetch underneath the v projection (ET ring, LAG=4)
  - phase 3 is one continuous stream over (qq, head, kj): 192 score tiles
    STt [kj,qi] grouped 3 per [128,1536] psum tile; ONE exp ACT op per
    group; yT accumulation consumes ET four groups behind so transient PE
    detours (proj tiles, normalize) never starve the ACT engine
  - yT_aug[65, qi] = [v_h | 1].T @ ET accumulated over kj; row 64 = denom
  - normalize: denom -> sbuf copy, reciprocal_approx_fast, stride-0 DMA
    broadcast, fused multiply deferred two groups (no DVE head-of-line)
  - out[qi, C] partial = yT (stationary) @ Wp rows, interleaved into the
    next block's stream at slot-parity-preserving spacing; the last four
    row-tiles split h0+h1 (early) / h2 (after the final normalize)
"""

import math

import numpy as np

import concourse.bacc as bacc
import concourse.bass as bass
import concourse.mybir as mybir
import concourse.tile as tile
from concourse import bass_utils

B, N, C, H, DH = 2, 2048, 768, 12, 64
NCORES = 8
CPG = 4                  # cores per batch group
HPC = H // CPG           # heads per core = 3
MYC = HPC * DH           # per-core feature width = 192
KC = C // 128            # contraction chunks = 6
NTT = N // 128           # token tiles = 16
QB = 512                 # qi block (psum bank width, fp32)
LAG = 4                  # ET ring depth: yt consumption trails ACT by LAG
F32 = mybir.dt.float32
MMDT = mybir.dt.float16  # matmul operand dtype
AF = mybir.ActivationFunctionType
OP = mybir.AluOpType

EXP_SHIFT = -4.0         # exp(s + EXP_SHIFT); cancels between num and denom


def _bcast_parts(ap, nparts):
    """Partition-stride-0 broadcast view of a [1, F] AP (DMA source only)."""
    return bass.AP(tensor=ap.tensor, offset=ap.offset,
                   ap=[[0, nparts]] + [list(d) for d in ap.ap[1:]])


def _emit(nc, tc, pools, aps):
    xT, wq, wk, wqk2, wv, wp, bq, bk, bv, out = (
        aps["xT"], aps["wq"], aps["wk"], aps["wqk2"], aps["wv"], aps["wp"],
        aps["bq"], aps["bk"], aps["bv"], aps["out"],
    )
    persist = pools["persist"]
    et_pool = pools["et"]
    small = pools["small"]
    ostage = pools["ostage"]
    dram_bc = pools["dram_bc"]

    # ---- persistent SBUF tensors ----
    xT_sb = persist.tile([128, KC * N], MMDT, tag="xT_sb")
    wq_sb = persist.tile([128, KC * 128], MMDT, tag="wq_sb")
    wk_sb = persist.tile([128, KC * 128], MMDT, tag="wk_sb")
    wqk2_sb = persist.tile([128, KC * 128], MMDT, tag="wqk2_sb")
    wv_sb = persist.tile([128, KC * MYC], MMDT, tag="wv_sb")
    wpA = persist.tile([128, C], MMDT, tag="wpA")
    wpB = persist.tile([64, C], MMDT, tag="wpB")
    bqA = persist.tile([128, 1], F32, tag="bqA")
    bqB = persist.tile([64, 1], F32, tag="bqB")
    bkA = persist.tile([128, 1], F32, tag="bkA")
    bkB = persist.tile([64, 1], F32, tag="bkB")
    bv_row = persist.tile([1, MYC], MMDT, tag="bv_row")
    ones = persist.tile([1, 128], MMDT, tag="ones")
    shift_col = persist.tile([128, 1], F32, tag="shift_col")
    # compact projections (h0 on parts 0:64, h1 on 64:128; h2 separate)
    qTA = persist.tile([128, N], MMDT, tag="qTA")
    kTA = persist.tile([128, N], MMDT, tag="kTA")
    # per-head partition-duplicated k/q for even/odd kj pair packing
    qTD = [persist.tile([128, N], MMDT, tag=f"qTD{h}", name=f"qTD{h}")
           for h in range(HPC)]
    kTD = [persist.tile([128, N], MMDT, tag=f"kTD{h}", name=f"kTD{h}")
           for h in range(HPC)]
    v_sb = persist.tile([128, NTT * HPC * 65], MMDT, tag="v_sb")
    yTA = persist.tile([128, N], MMDT, tag="yTA")
    yTB = persist.tile([64, N], MMDT, tag="yTB")

    # ---- input DMAs: qk-proj operands first so phase 1 starts ASAP ----
    for kc in range(KC):
        nc.sync.dma_start(out=xT_sb[:, kc * N:(kc + 1) * N],
                          in_=xT[kc * 128:(kc + 1) * 128, :])
        nc.sync.dma_start(out=wq_sb[:, kc * 128:(kc + 1) * 128],
                          in_=wq[kc * 128:(kc + 1) * 128, :])
        nc.sync.dma_start(out=wk_sb[:, kc * 128:(kc + 1) * 128],
                          in_=wk[kc * 128:(kc + 1) * 128, :])
        nc.sync.dma_start(out=wqk2_sb[:, kc * 128:(kc + 1) * 128],
                          in_=wqk2[kc * 128:(kc + 1) * 128, :])
    nc.sync.dma_start(out=bqA, in_=bq[0:128, :])
    nc.sync.dma_start(out=bqB, in_=bq[128:MYC, :])
    nc.sync.dma_start(out=bkA, in_=bk[0:128, :])
    nc.sync.dma_start(out=bkB, in_=bk[128:MYC, :])
    for kc in range(KC):
        nc.sync.dma_start(out=wv_sb[:, kc * MYC:(kc + 1) * MYC],
                          in_=wv[kc * 128:(kc + 1) * 128, :])
    nc.sync.dma_start(out=bv_row, in_=bv)
    nc.sync.dma_start(out=wpA, in_=wp[0:128, :])
    nc.sync.dma_start(out=wpB, in_=wp[128:MYC, :])
    ones_f32 = persist.tile([128, 1], F32, tag="ones_f32")
    ones_row_f32 = persist.tile([1, 128], F32, tag="ones_row_f32")
    nc.vector.memset(ones_f32, 1.0)
    nc.vector.memset(ones_row_f32, 1.0)
    nc.vector.tensor_copy(out=ones, in_=ones_row_f32)
    nc.vector.memset(shift_col, EXP_SHIFT)

    # ---- phase 1: q/k/h2-combined projections (M=128 passes) ----
    with tc.tile_pool(name="ps_qk", bufs=2, space="PSUM") as ps_qk:
        pssQ = [ps_qk.tile([128, QB], F32, tag="ps_qkA", bufs=4,
                           name=f"ps_q{_i}") for _i in range(N // QB)]
        pssK = [ps_qk.tile([128, QB], F32, tag="ps_qkB", bufs=4,
                           name=f"ps_k{_i}") for _i in range(N // QB)]
        # q and k interleaved per kc chunk so matmul consumption stays
        # behind the xT DMA supply
        for kc in range(KC):
            for nt in range(N // QB):
                nc.tensor.matmul(
                    pssQ[nt], wq_sb[:, kc * 128:(kc + 1) * 128],
                    xT_sb[:, kc * N + nt * QB: kc * N + nt * QB + QB],
                    start=(kc == 0), stop=(kc == KC - 1))
            for nt in range(N // QB):
                nc.tensor.matmul(
                    pssK[nt], wk_sb[:, kc * 128:(kc + 1) * 128],
                    xT_sb[:, kc * N + nt * QB: kc * N + nt * QB + QB],
                    start=(kc == 0), stop=(kc == KC - 1))
        for nt in range(N // QB):
            nc.vector.tensor_scalar(
                out=qTA[:, nt * QB:(nt + 1) * QB], in0=pssQ[nt],
                scalar1=bqA, scalar2=None, op0=OP.add)
            nc.vector.tensor_scalar(
                out=kTA[:, nt * QB:(nt + 1) * QB], in0=pssK[nt],
                scalar1=bkA, scalar2=None, op0=OP.add)
        # combined h2 pass: psum rows 0:64 = q-h2, rows 64:128 = k-h2
        pss2 = [ps_qk.tile([128, QB], F32, tag="ps_qkA", bufs=4,
                           name=f"ps_2{_i}") for _i in range(N // QB)]
        for kc in range(KC):
            for nt in range(N // QB):
                nc.tensor.matmul(
                    pss2[nt], wqk2_sb[:, kc * 128:(kc + 1) * 128],
                    xT_sb[:, kc * N + nt * QB: kc * N + nt * QB + QB],
                    start=(kc == 0), stop=(kc == KC - 1))
        for nt in range(N // QB):
            nc.vector.tensor_scalar(
                out=qTD[2][0:64, nt * QB:(nt + 1) * QB], in0=pss2[nt][0:64, :],
                scalar1=bqB, scalar2=None, op0=OP.add)
            nc.vector.tensor_scalar(
                out=kTD[2][0:64, nt * QB:(nt + 1) * QB],
                in0=pss2[nt][64:128, :],
                scalar1=bkB, scalar2=None, op0=OP.add)
        # duplicate each head's 64 dims onto both partition halves (DMA)
        nc.sync.dma_start(out=qTD[2][64:128, :], in_=qTD[2][0:64, :])
        nc.sync.dma_start(out=kTD[2][64:128, :], in_=kTD[2][0:64, :])
        nc.sync.dma_start(out=qTD[0][0:64, :], in_=qTA[0:64, :])
        nc.sync.dma_start(out=qTD[0][64:128, :], in_=qTA[0:64, :])
        nc.sync.dma_start(out=qTD[1][0:64, :], in_=qTA[64:128, :])
        nc.sync.dma_start(out=qTD[1][64:128, :], in_=qTA[64:128, :])
        nc.sync.dma_start(out=kTD[0][0:64, :], in_=kTA[0:64, :])
        nc.sync.dma_start(out=kTD[0][64:128, :], in_=kTA[0:64, :])
        nc.sync.dma_start(out=kTD[1][0:64, :], in_=kTA[64:128, :])
        nc.sync.dma_start(out=kTD[1][64:128, :], in_=kTA[64:128, :])

    # ---- phases 2+3: v projection + score stream share the PSUM pools ----
    def vh_ap(kj, h):
        base = (kj * HPC + h) * 65
        return v_sb[:, base:base + 65]

    # normalize phase 1: denom row -> sbuf, fast reciprocal, broadcast
    # via DRAM (stride-0 partition source requires linear memory).  The
    # fused multiply (phase 2) is DEFERRED two groups so its bc-DMA wait
    # never head-of-line-blocks the DVE queue.
    def norm_start(yt, h, qq):
        den = small.tile([1, QB], F32, tag="den")
        nc.vector.tensor_copy(out=den, in_=yt[64:65, :])
        rec = small.tile([1, QB], F32, tag="rec")
        nc.vector.reciprocal_approx_fast(rec, den)
        dr = dram_bc.tile([1, QB], F32, tag="dr", name=f"dr{h}_{qq}")
        nc.sync.dma_start(out=dr, in_=rec)
        bc = small.tile([64, QB], F32, tag="bc_sb")
        nc.sync.dma_start(out=bc, in_=_bcast_parts(dr, 64))
        return (yt, bc, h, qq)

    def norm_finish(state):
        yt, bc, h, qq = state
        q0 = qq * QB
        ydst = yTA[0:64, :] if h == 0 else (
            yTA[64:128, :] if h == 1 else yTB[0:64, :])
        nc.vector.scalar_tensor_tensor(
            out=ydst[:, q0:q0 + QB], in0=yt[0:64, :], scalar=1.0, in1=bc,
            op0=OP.mult, op1=OP.mult,
        )

    def proj_full(ps_st, qt):
        # one output row-tile [128, C]; psum borrowed from the st pool
        stt = ps_st.tile([128, 3 * QB], F32, tag="st", name=f"pj{qt}")
        ob = ostage.tile([128, C], F32, tag="ob", name=f"ob{qt}")
        for nb in range(2):
            po = stt[:, nb * QB: nb * QB + 384]
            nc.tensor.matmul(po, yTA[:, qt * 128:(qt + 1) * 128],
                             wpA[:, nb * 384:(nb + 1) * 384],
                             start=True, stop=False)
            nc.tensor.matmul(po, yTB[0:64, qt * 128:(qt + 1) * 128],
                             wpB[0:64, nb * 384:(nb + 1) * 384],
                             start=False, stop=True)
            nc.vector.tensor_copy(out=ob[:, nb * 384:(nb + 1) * 384],
                                  in_=po)
        nc.sync.dma_start(out=out[qt * 128:(qt + 1) * 128, :], in_=ob)

    stream = [(qq, h, kj)
              for qq in range(4) for h in range(HPC) for kj in range(NTT)]
    NG = len(stream) // 3  # 64 groups of 3 score tiles

    # proj for block qq interleaved into block qq+1's stream; 2-group
    # spacing keeps st-pool slot parity (adjacent insertions would share a
    # slot and serialize on the DVE po copies); keyed by CONSUMED group
    proj_at = {}
    for qq in range(3):
        for j, goff in enumerate((8, 10, 12, 14)):
            proj_at.setdefault((qq + 1) * 16 + goff, []).append(qq * 4 + j)

    def emit_st_group(ps_st, g):
        entries = [stream[3 * g + j] for j in range(3)]
        st = ps_st.tile([128, 3 * QB], F32, tag="st", name=f"st{g}")
        for j, (qq, h, kj) in enumerate(entries):
            lo = 0 if kj % 2 == 0 else 64
            nc.tensor.matmul(
                st[:, j * QB:(j + 1) * QB],
                kTD[h][lo:lo + 64, kj * 128:(kj + 1) * 128],
                qTD[h][lo:lo + 64, qq * QB:(qq + 1) * QB],
                start=True, stop=True,
            )
        et = et_pool.tile([128, 3 * QB], MMDT, tag="et", name=f"et{g}")
        nc.scalar.activation(et, st, AF.Exp, bias=shift_col[:, :])
        return (et, entries)

    with tc.tile_pool(name="ps_st", bufs=2, space="PSUM") as ps_st, \
         tc.tile_pool(name="ps_yt", bufs=2, space="PSUM") as ps_yt:
        ring = []
        # prefetch the first LAG score groups: their exp runs under v-proj
        # (2 groups fill the st slots; 2 more slot in after a few v tiles
        # so the PE never queues behind an ACT wait)
        def emit_v(nt):
            ps = ps_yt.tile([128, MYC], F32, tag="yt", name=f"ps_v{nt}")
            for kc in range(KC):
                nc.tensor.matmul(
                    ps,
                    xT_sb[:, kc * N + nt * 128: kc * N + nt * 128 + 128],
                    wv_sb[:, kc * MYC:(kc + 1) * MYC],
                    start=(kc == 0), stop=False,
                )
            nc.tensor.matmul(ps, ones[0:1, 0:128], bv_row,
                             start=False, stop=True)
            for h in range(HPC):
                base = (nt * HPC + h) * 65
                nc.vector.tensor_copy(out=v_sb[:, base:base + 64],
                                      in_=ps[:, h * 64:(h + 1) * 64])
                nc.vector.tensor_copy(out=v_sb[:, base + 64:base + 65],
                                      in_=ones_f32)

        ring.append(emit_st_group(ps_st, 0))
        ring.append(emit_st_group(ps_st, 1))
        for nt in range(4):
            emit_v(nt)
        ring.append(emit_st_group(ps_st, 2))
        ring.append(emit_st_group(ps_st, 3))
        for nt in range(4, NTT):
            emit_v(nt)

        # ---- phase 3 main loop ----
        yt_cur = {}
        pending = []   # (due consumed-group, norm state)
        pjAB = []
        for gi in range(LAG, NG + LAG):
            if gi < NG:
                ring.append(emit_st_group(ps_st, gi))
            cg = gi - LAG
            pet, pentries = ring.pop(0)
            for j, (qq, h, kj) in enumerate(pentries):
                if kj == 0:
                    yt_cur[(qq, h)] = ps_yt.tile([65, QB], F32, tag="yt",
                                                 name=f"yt{qq}_{h}")
                nc.tensor.matmul(yt_cur[(qq, h)], vh_ap(kj, h),
                                 pet[:, j * QB:(j + 1) * QB],
                                 start=(kj == 0), stop=(kj == NTT - 1))
                if kj == NTT - 1:
                    pending.append(
                        (cg + 2, norm_start(yt_cur.pop((qq, h)), h, qq)))
            while pending and pending[0][0] <= cg:
                norm_finish(pending.pop(0)[1])
            for qt in proj_at.get(cg + 1, []):
                proj_full(ps_st, qt)
        # final block's h0+h1 proj contribution emitted after every other
        # st-pool user (the partA tiles hold both slots until partB):
        # 8 bank-aligned po regions (3+3 in the st slots, 2 from yt pool)
        stA = ps_st.tile([128, 3 * QB], F32, tag="st", name="pjtA")
        stB = ps_st.tile([128, 3 * QB], F32, tag="st", name="pjtB")
        poY = [ps_yt.tile([128, 384], F32, tag="yt", name=f"poY{_i}")
               for _i in range(2)]
        for p in range(8):
            qt, nb = 12 + p // 2, p % 2
            if p < 3:
                po = stA[:, p * QB: p * QB + 384]
            elif p < 6:
                po = stB[:, (p - 3) * QB: (p - 3) * QB + 384]
            else:
                po = poY[p - 6][:, 0:384]
            pjAB.append(po)
            nc.tensor.matmul(po, yTA[:, qt * 128:(qt + 1) * 128],
                             wpA[:, nb * 384:(nb + 1) * 384],
                             start=True, stop=False)
        while pending:
            norm_finish(pending.pop(0)[1])
        for qx in range(4):
            qt = 12 + qx
            ob = ostage.tile([128, C], F32, tag="ob", name=f"ob{qt}")
            for nb in range(2):
                po = pjAB[qx * 2 + nb]
                nc.tensor.matmul(po, yTB[0:64, qt * 128:(qt + 1) * 128],
                                 wpB[0:64, nb * 384:(nb + 1) * 384],
                                 start=False, stop=True)
                nc.vector.tensor_copy(out=ob[:, nb * 384:(nb + 1) * 384],
                                      in_=po)
            nc.sync.dma_start(out=out[qt * 128:(qt + 1) * 128, :], in_=ob)


def _build_program():
    nc = bacc.Bacc("TRN2", target_bir_lowering=False, debug=False,
                   num_devices=NCORES)
    aps = {
        "xT": nc.dram_tensor("xT", [C, N], MMDT, kind="ExternalInput").ap(),
        "wq": nc.dram_tensor("wq", [C, 128], MMDT, kind="ExternalInput").ap(),
        "wk": nc.dram_tensor("wk", [C, 128], MMDT, kind="ExternalInput").ap(),
        "wqk2": nc.dram_tensor("wqk2", [C, 128], MMDT,
                               kind="ExternalInput").ap(),
        "wv": nc.dram_tensor("wv", [C, MYC], MMDT, kind="ExternalInput").ap(),
        "wp": nc.dram_tensor("wp", [MYC, C], MMDT, kind="ExternalInput").ap(),
        "bq": nc.dram_tensor("bq", [MYC, 1], F32, kind="ExternalInput").ap(),
        "bk": nc.dram_tensor("bk", [MYC, 1], F32, kind="ExternalInput").ap(),
        "bv": nc.dram_tensor("bv", [1, MYC], MMDT, kind="ExternalInput").ap(),
        "out": nc.dram_tensor("out", [N, C], F32, kind="ExternalOutput").ap(),
    }
    with tile.TileContext(nc) as tc:
        import contextlib
        with contextlib.ExitStack() as ctx:
            pools = {
                "persist": ctx.enter_context(tc.tile_pool(name="persist", bufs=1)),
                "et": ctx.enter_context(tc.tile_pool(name="et", bufs=LAG + 1)),
                "small": ctx.enter_context(tc.tile_pool(name="small", bufs=3)),
                "ostage": ctx.enter_context(tc.tile_pool(name="ostage", bufs=2)),
                "dram_bc": ctx.enter_context(
                    tc.tile_pool(name="dram_bc", bufs=2, space="DRAM")),
            }
            _emit(nc, tc, pools, aps)
    nc.compile()
    return nc


_PROGRAM_CACHE = {}


def _get_program():
    if "nc" not in _PROGRAM_CACHE:
        _PROGRAM_CACHE["nc"] = _build_program()
    return _PROGRAM_CACHE["nc"]


def make_in_maps(x, Wq, bq, Wk, bk, Wv, bv, Wp, bp):
    scale = 1.0 / math.sqrt(DH)
    xTb = [np.ascontiguousarray(x[b].T) for b in range(B)]
    wire = mybir.dt.np(MMDT)
    in_maps = []
    for c in range(NCORES):
        b, hg = c // CPG, c % CPG
        cols = slice(hg * MYC, (hg + 1) * MYC)
        wqc = Wq[:, cols] * np.float32(scale)
        wkc = Wk[:, cols]
        in_maps.append({
            "xT": xTb[b].astype(wire),
            "wq": np.ascontiguousarray(wqc[:, 0:128]).astype(wire),
            "wk": np.ascontiguousarray(wkc[:, 0:128]).astype(wire),
            "wqk2": np.ascontiguousarray(
                np.concatenate([wqc[:, 128:192], wkc[:, 128:192]],
                               axis=1)).astype(wire),
            "wv": np.ascontiguousarray(Wv[:, cols]).astype(wire),
            "wp": np.ascontiguousarray(Wp[cols, :]).astype(wire),
            "bq": (bq[cols] * np.float32(scale)).reshape(MYC, 1).copy(),
            "bk": bk[cols].reshape(MYC, 1).copy(),
            "bv": bv[cols].reshape(1, MYC).astype(wire),
        })
    return in_maps


def assemble(results, bp):
    out = np.empty((B, N, C), np.float32)
    for b in range(B):
        acc = results[b * CPG]["out"].astype(np.float64)
        for c in range(b * CPG + 1, (b + 1) * CPG):
            acc = acc + results[c]["out"]
        out[b] = (acc + bp.astype(np.float64)).astype(np.float32)
    return out


def kernel(x, Wq, bq, Wk, bk, Wv, bv, Wp, bp, **extra_kwargs):
    x = np.asarray(x, np.float32)
    Wq = np.asarray(Wq, np.float32)
    Wk = np.asarray(Wk, np.float32)
    Wv = np.asarray(Wv, np.float32)
    Wp = np.asarray(Wp, np.float32)
    bq = np.asarray(bq, np.float32)
    bk = np.asarray(bk, np.float32)
    bv = np.asarray(bv, np.float32)
    bp = np.asarray(bp, np.float32)

    nc = _get_program()
    in_maps = make_in_maps(x, Wq, bq, Wk, bk, Wv, bv, Wp, bp)
    res = bass_utils.run_bass_kernel_spmd(nc, in_maps,
                                          core_ids=list(range(NCORES)))
    return assemble(res.results, bp)
